# revision 1
# baseline (speedup 1.0000x reference)
"""GCN layer (x@W1 -> relu -> @W2 -> weighted scatter-add over edges) on 8 TRN2 cores.

Strategy (two launches, 8-way SPMD):
  L1: row-shard x across cores; each core computes its shard of
      support = relu(x@W1+b1)@W2 via TensorE (all fp32), writes its
      [N/8, 64] table shard. Host concatenates shards -> full table.
  L2: edges partitioned by destination shard (core = dst // (N/8)).
      Per core: dma_gather (GPSIMD extended inst, mlp Q7 library) fetches
      256B table rows for each edge (src-indexed, 4 int16 index blocks),
      DVE builds a weighted one-hot per 128-edge chunk (tensor_scalar
      is_equal+mult against an iota matrix), TensorE accumulates
      agg.T[64, dst_tile] += msgs.T @ onehot in PSUM, bias b2 added at
      PSUM evacuation. Host transposes/concats shards.

All floating-point math happens on device in fp32; the host only shards,
sorts edge indices, and concatenates outputs.
"""

import sys

if "/opt/trn_rl_repo" not in sys.path:
    sys.path.insert(0, "/opt/trn_rl_repo")

import numpy as np

import concourse.bass as bass
import concourse.tile as tile
from concourse import library_config, mybir
from concourse.bass_utils import run_bass_kernel_spmd
from concourse.library_overlay import lower_extended_insts

F32 = mybir.dt.float32
I16 = mybir.dt.int16

NCORES = 8
NBLK = 4  # int16 gather-index blocks (N/NBLK must be < 32768)
R_TILES = 4  # dst tiles per gather range

MAX_WAITS = 1  # this walrus build rejects >1 semaphore wait per instruction


def _split_excess_waits(nc, max_waits=MAX_WAITS):
    """Move excess sem-waits onto injected same-engine NOPs placed before the
    over-subscribed instruction (same-engine program order keeps semantics)."""
    uid = 0
    for f in nc.m.functions:
        for bb in f.blocks:
            il = bb.instructions
            new_il = []
            for inst in il:
                si = inst.sync_info
                waits = list(si.on_wait) if si and si.on_wait else []
                if len(waits) > max_waits:
                    excess, keep = waits[:-max_waits], waits[-max_waits:]
                    for j in range(0, len(excess), max_waits):
                        grp = excess[j : j + max_waits]
                        nop = mybir.InstNoOp(name=f"I-waitsplit-{uid}", ins=[], outs=[])
                        uid += 1
                        nop.engine = inst.engine
                        nop.sync_info = mybir.SyncInfo(on_wait=grp, on_update=[])
                        nc.register_instruction(nop, overwrite=True)
                        new_il.append(nop)
                    si.on_wait = keep
                new_il.append(inst)
            il[:] = new_il


def _finalize(nc):
    lower_extended_insts(nc)
    _split_excess_waits(nc)


# ---------------------------------------------------------------- L1: matmuls


def _build_l1(n_nodes, nfeat, nhid, ncls):
    shard = n_nodes // NCORES
    nc = bass.Bass()
    xT = nc.dram_tensor("xT", [nfeat, shard], F32, kind="ExternalInput")
    W1 = nc.dram_tensor("W1", [nfeat, nhid], F32, kind="ExternalInput")
    b1 = nc.dram_tensor("b1", [nhid, 1], F32, kind="ExternalInput")
    W2 = nc.dram_tensor("W2", [nhid, ncls], F32, kind="ExternalInput")
    table = nc.dram_tensor("table", [shard, ncls], F32, kind="ExternalOutput")

    kchunks = nfeat // 128
    assert nfeat % 128 == 0 and nhid == 128
    NCHW = 512  # node columns per h chunk
    nch = (shard + NCHW - 1) // NCHW
    ntiles = (shard + 127) // 128

    with tile.TileContext(nc) as tc:
        with (
            tc.tile_pool(name="const", bufs=1) as constp,
            tc.tile_pool(name="xbuf", bufs=3) as xbuf,
            tc.tile_pool(name="hbuf", bufs=1) as hbuf,
            tc.tile_pool(name="obuf", bufs=4) as obuf,
            tc.tile_pool(name="psh", bufs=4, space="PSUM") as psh,
            tc.tile_pool(name="pss", bufs=4, space="PSUM") as pss,
        ):
            w1s = constp.tile([128, kchunks, nhid], F32)
            nc.sync.dma_start(
                out=w1s[:], in_=W1[:].rearrange("(k p) h -> p k h", p=128)
            )
            w2s = constp.tile([128, ncls], F32)
            nc.sync.dma_start(out=w2s[:], in_=W2[:])
            b1s = constp.tile([128, 1], F32)
            nc.sync.dma_start(out=b1s[:], in_=b1[:])

            hT = hbuf.tile([128, shard], F32)  # resident h.T (fp32)
            for j in range(nch):
                j0 = j * NCHW
                nsz = min(NCHW, shard - j0)
                xt = xbuf.tile([128, kchunks, NCHW], F32, tag="xt")
                nc.sync.dma_start(
                    out=xt[:, :, :nsz],
                    in_=xT[:, j0 : j0 + nsz].rearrange("(k p) n -> p k n", p=128),
                )
                ph = psh.tile([128, NCHW], F32)
                for k in range(kchunks):
                    nc.tensor.matmul(
                        ph[:, :nsz],
                        w1s[:, k, :],
                        xt[:, k, :nsz],
                        start=(k == 0),
                        stop=(k == kchunks - 1),
                    )
                nc.scalar.activation(
                    hT[:, j0 : j0 + nsz],
                    ph[:, :nsz],
                    mybir.ActivationFunctionType.Relu,
                    bias=b1s[:],
                    scale=1.0,
                )
            for t in range(ntiles):
                t0 = t * 128
                msz = min(128, shard - t0)
                ps = pss.tile([128, ncls], F32)
                nc.tensor.matmul(
                    ps[:msz, :], hT[:, t0 : t0 + msz], w2s[:], start=True, stop=True
                )
                ob = obuf.tile([128, ncls], F32)
                nc.vector.tensor_copy(ob[:msz, :], ps[:msz, :])
                nc.sync.dma_start(out=table[t0 : t0 + msz, :], in_=ob[:msz, :])

    _finalize(nc)
    return nc


# ------------------------------------------------- edge schedule (host side)


def _edge_schedule(src, dst, ew, n_nodes, shard):
    """Partition edges by destination shard, sort by (dst tile, src block),
    build the SPMD-common gather/compute schedule (max counts over cores) and
    each core's index/weight streams laid into that skeleton.

    Returns (schedule, percore, dims).
    """
    blk = n_nodes // NBLK
    ntiles = (shard + 127) // 128
    core_of = dst // shard

    percore_edges = []
    cnt_all = np.zeros((NCORES, ntiles, NBLK), np.int64)
    for c in range(NCORES):
        m = core_of == c
        s = src[m]
        d = dst[m] - c * shard
        w = ew[m]
        tl = d // 128
        bl = s // blk
        order = np.lexsort((bl, tl))
        s, d, w, tl, bl = s[order], d[order], w[order], tl[order], bl[order]
        cnt = np.zeros((ntiles, NBLK), np.int64)
        np.add.at(cnt, (tl, bl), 1)
        cnt_all[c] = cnt
        run_off = np.zeros(ntiles * NBLK + 1, np.int64)
        np.cumsum(cnt.reshape(-1), out=run_off[1:])
        percore_edges.append((s, d, w, cnt, run_off))

    # SPMD skeleton: chunks per (tile, block) = ceil(max-over-cores / 128)
    nr_tb = (cnt_all.max(axis=0) + 127) // 128
    for t in range(ntiles):
        if nr_tb[t].sum() == 0:
            nr_tb[t, 0] = 1  # keep every tile non-empty

    nranges = (ntiles + R_TILES - 1) // R_TILES
    schedule = []
    icol_off = 0
    chunk_off = 0
    gmax = 1
    run_loc = {}  # (t, b) -> (icol, chunkcol, nchunks)
    for r in range(nranges):
        tlo, thi = r * R_TILES, min(ntiles, r * R_TILES + R_TILES)
        gathers = []
        for b in range(NBLK):
            nch_rb = int(nr_tb[tlo:thi, b].sum())
            if nch_rb == 0:
                continue
            off = 0
            for t in range(tlo, thi):
                if nr_tb[t, b]:
                    run_loc[(t, b)] = (
                        icol_off + off * 8,
                        chunk_off + off,
                        int(nr_tb[t, b]),
                    )
                    off += int(nr_tb[t, b])
            gathers.append(
                dict(
                    b=b,
                    nidx=nch_rb * 128,
                    icol=icol_off,
                    chunk0=chunk_off,
                    nchunks=nch_rb,
                    buf_off=0,
                )
            )
            gmax = max(gmax, nch_rb)
            icol_off += nch_rb * 8  # 128 idx per chunk = 8 cols of 16
            chunk_off += nch_rb
        tiles = []
        for t in range(tlo, thi):
            msz = min(128, shard - t * 128)
            runs = []
            for g in gathers:
                b = g["b"]
                if nr_tb[t, b]:
                    icol, chcol, nchk = run_loc[(t, b)]
                    runs.append((b, chcol - g["chunk0"], nchk))
            tiles.append(dict(t=t, msz=msz, runs=runs))
        schedule.append(dict(gathers=gathers, tiles=tiles))

    icols = max(icol_off, 16)
    tch = max(chunk_off, 1)

    percore = []
    for c in range(NCORES):
        s, d, w, cnt, run_off = percore_edges[c]
        idx_flat = np.zeros(tch * 128, np.int16)
        dst_flat = np.zeros(tch * 128, np.float32)
        w_flat = np.zeros(tch * 128, np.float32)
        for (t, b), (icol, chcol, nchk) in run_loc.items():
            n_real = int(cnt[t, b])
            if n_real == 0:
                continue
            i0 = int(run_off[t * NBLK + b])
            o0 = chcol * 128
            idx_flat[o0 : o0 + n_real] = (s[i0 : i0 + n_real] - b * blk).astype(
                np.int16
            )
            dst_flat[o0 : o0 + n_real] = (d[i0 : i0 + n_real] - t * 128).astype(
                np.float32
            )
            w_flat[o0 : o0 + n_real] = w[i0 : i0 + n_real]
        idx16 = np.tile(idx_flat.reshape(-1, 16).T, (8, 1))  # [128, tch*8]
        if idx16.shape[1] < icols:
            idx16 = np.pad(idx16, ((0, 0), (0, icols - idx16.shape[1])))
        dstw = dst_flat.reshape(tch, 128).T.copy()
        wmat = w_flat.reshape(tch, 128).T.copy()
        percore.append(dict(idx=np.ascontiguousarray(idx16), dstw=dstw, wmat=wmat))

    fp = hash((nr_tb.tobytes(), shard, n_nodes))
    dims = dict(icols=icols, tch=tch, gmax=gmax, fingerprint=fp)
    return schedule, percore, dims


# ---------------------------------------------------------------- L2: edges


def _build_l2(n_nodes, ncls, shard, schedule, dims):
    blk = n_nodes // NBLK
    icols, tch, gmax = dims["icols"], dims["tch"], dims["gmax"]
    nc = bass.Bass()
    table = nc.dram_tensor("table", [n_nodes, ncls], F32, kind="ExternalInput")
    idxs = nc.dram_tensor("idxs", [128, icols], I16, kind="ExternalInput")
    dstw = nc.dram_tensor("dstw", [128, tch], F32, kind="ExternalInput")
    wmat = nc.dram_tensor("wmat", [128, tch], F32, kind="ExternalInput")
    b2t = nc.dram_tensor("b2t", [ncls, 1], F32, kind="ExternalInput")
    aggT = nc.dram_tensor("aggT", [ncls, shard], F32, kind="ExternalOutput")

    iota_np = np.tile(np.arange(128, dtype=np.float32), (128, 1))
    iota_t = nc.inline_tensor(iota_np, "iota")

    from contextlib import ExitStack

    with tile.TileContext(nc) as tc, ExitStack() as es:
        nidx_reg = es.enter_context(nc.gpsimd.register("nidx_reg"))
        with (
            tc.tile_pool(name="const", bufs=1) as constp,
            tc.tile_pool(name="idxp", bufs=2) as idxp,
            tc.tile_pool(name="gp", bufs=2) as gp,
            tc.tile_pool(name="ohp", bufs=6) as ohp,
            tc.tile_pool(name="evp", bufs=4) as evp,
            tc.tile_pool(name="psp", bufs=6, space="PSUM") as psp,
        ):
            nc.gpsimd.load_library(library_config.mlp)
            iota_s = constp.tile([128, 128], F32)
            nc.sync.dma_start(out=iota_s[:], in_=iota_t[:])
            b2s = constp.tile([ncls, 1], F32)
            nc.sync.dma_start(out=b2s[:], in_=b2t[:])
            dstw_s = constp.tile([128, tch], F32)
            nc.sync.dma_start(out=dstw_s[:], in_=dstw[:])
            wmat_s = constp.tile([128, tch], F32)
            nc.sync.dma_start(out=wmat_s[:], in_=wmat[:])

            for rng in schedule:
                gathers = rng["gathers"]
                gbufs = {}
                if gathers:
                    icol0 = gathers[0]["icol"]
                    icoln = gathers[-1]["icol"] + gathers[-1]["nidx"] // 16
                    ib = idxp.tile([128, icoln - icol0], I16, tag="idx")
                    nc.sync.dma_start(out=ib[:], in_=idxs[:, icol0:icoln])
                for g in gathers:
                    b = g["b"]
                    gb = gp.tile([128, gmax, ncls], F32, tag=f"g{b}")
                    gbufs[b] = g
                    g["tile"] = gb
                    # SWDGE single-packet ring: <= 1024 indices per gather
                    GCAP = 8  # chunks per sub-gather
                    for s0 in range(0, g["nchunks"], GCAP):
                        s1 = min(g["nchunks"], s0 + GCAP)
                        nidx = (s1 - s0) * 128
                        ic = g["icol"] - icol0 + s0 * 8
                        nc.gpsimd.reg_mov(nidx_reg, nidx)
                        nc.gpsimd.dma_gather(
                            gb[:, s0:s1, :],
                            table[b * blk : (b + 1) * blk, :],
                            ib[:, ic : ic + nidx // 16],
                            nidx,
                            nidx_reg,
                            ncls,
                        )
                for tt in rng["tiles"]:
                    t, msz, runs = tt["t"], tt["msz"], tt["runs"]
                    ps = psp.tile([ncls, 128], F32, tag="ps")
                    nchunks_t = sum(nr for (_, _, nr) in runs)
                    ci = 0
                    for b, c0, nr in runs:
                        g = gbufs[b]
                        gb = g["tile"]
                        for j in range(nr):
                            col = g["chunk0"] + c0 + j
                            oh = ohp.tile([128, 128], F32, tag="oh")
                            nc.vector.tensor_scalar(
                                oh[:],
                                iota_s[:],
                                dstw_s[:, col : col + 1],
                                wmat_s[:, col : col + 1],
                                mybir.AluOpType.is_equal,
                                mybir.AluOpType.mult,
                            )
                            nc.tensor.matmul(
                                ps[:, :msz],
                                gb[:, c0 + j, :],
                                oh[:, :msz],
                                start=(ci == 0),
                                stop=(ci == nchunks_t - 1),
                            )
                            ci += 1
                    ev = evp.tile([ncls, 128], F32, tag="ev")
                    nc.vector.tensor_scalar_add(ev[:, :msz], ps[:, :msz], b2s[:])
                    nc.sync.dma_start(
                        out=aggT[:, t * 128 : t * 128 + msz], in_=ev[:, :msz]
                    )

    _finalize(nc)
    return nc


# ------------------------------------------------------------------- driver

_CACHE = {}
LAST_TIMES = {}


def _timed_run(name, nc, in_maps, core_ids):
    import time as _time

    t0 = _time.time()
    res = run_bass_kernel_spmd(nc, in_maps, core_ids)
    LAST_TIMES[name] = _time.time() - t0
    return res


def kernel(x, W1, b1, W2, b2, edge_index, edge_weight):
    x = np.asarray(x, np.float32)
    W1 = np.asarray(W1, np.float32)
    b1 = np.asarray(b1, np.float32)
    W2 = np.asarray(W2, np.float32)
    b2 = np.asarray(b2, np.float32)
    edge_index = np.asarray(edge_index)
    edge_weight = np.asarray(edge_weight, np.float32)

    n_nodes, nfeat = x.shape
    ncls = W2.shape[1]
    shard = n_nodes // NCORES
    core_ids = list(range(NCORES))

    # ---- L1: support table ----
    key1 = ("l1", n_nodes, nfeat, W1.shape[1], ncls)
    if key1 not in _CACHE:
        _CACHE[key1] = _build_l1(n_nodes, nfeat, W1.shape[1], ncls)
    nc1 = _CACHE[key1]

    xT = np.ascontiguousarray(x.T)
    in_maps1 = [
        {
            "xT": np.ascontiguousarray(xT[:, c * shard : (c + 1) * shard]),
            "W1": W1,
            "b1": np.ascontiguousarray(b1.reshape(-1, 1)),
            "W2": W2,
        }
        for c in core_ids
    ]
    res1 = _timed_run("l1", nc1, in_maps1, core_ids)
    table = np.ascontiguousarray(
        np.concatenate([res1.results[c]["table"] for c in core_ids], axis=0)
    )

    # ---- host edge preprocessing ----
    src = edge_index[0].astype(np.int64)
    dst = edge_index[1].astype(np.int64)
    ekey = ("sched", n_nodes, shard, edge_index.shape[1])
    if ekey in _CACHE and _CACHE[ekey][0] is not None:
        fph, schedule, percore, dims = _CACHE[ekey]
        if fph != hash(edge_index.tobytes()):
            schedule = None
    else:
        schedule = None
    if schedule is None:
        schedule, percore, dims = _edge_schedule(
            src, dst, edge_weight, n_nodes, shard
        )
        _CACHE[ekey] = (hash(edge_index.tobytes()), schedule, percore, dims)

    key2 = ("l2", n_nodes, ncls, shard, dims["fingerprint"])
    if key2 not in _CACHE:
        _CACHE[key2] = _build_l2(n_nodes, ncls, shard, schedule, dims)
    nc2 = _CACHE[key2]

    b2c = np.ascontiguousarray(b2.reshape(-1, 1))
    in_maps2 = [
        {
            "table": table,
            "idxs": percore[c]["idx"],
            "dstw": percore[c]["dstw"],
            "wmat": percore[c]["wmat"],
            "b2t": b2c,
        }
        for c in core_ids
    ]
    res2 = _timed_run("l2", nc2, in_maps2, core_ids)
    out = np.concatenate(
        [np.ascontiguousarray(res2.results[c]["aggT"].T) for c in core_ids], axis=0
    )
    return out



# revision 5
# speedup vs baseline: 2.4024x; 2.4024x over previous
"""GCN layer (x@W1 -> relu -> @W2 -> weighted scatter-add over edges) on 8 TRN2 cores.

Strategy (two launches, 8-way SPMD):
  L1: row-shard x across cores; each core computes its shard of
      support = relu(x@W1+b1)@W2 via TensorE (fp32 accumulate), writes its
      [N/8, 128] bf16 table shard (64 features + 64-byte pad -> 256B rows,
      the dma_gather minimum row size). Host concatenates shards.
  L2: edges partitioned by destination shard (core = dst // (N/8)).
      Per core: dma_gather (GPSIMD SWDGE, 4 queues, 512-idx calls, deep
      buffering) fetches 256B table rows per edge; DVE scales messages by
      edge weight (batched tensor_tensor) and builds 0/1 one-hot matrices
      (batched is_equal against an iota tile); TensorE accumulates
      agg.T[64, dst_tile] += msg.T @ onehot in PSUM; bias b2 added at PSUM
      evacuation. Host transposes/concats shards.

All floating-point math happens on device; the host only shards, sorts edge
indices, and concatenates outputs.
"""

import sys

if "/opt/trn_rl_repo" not in sys.path:
    sys.path.insert(0, "/opt/trn_rl_repo")

import numpy as np

import concourse.bass as bass
import concourse.tile as tile
from concourse import library_config, mybir
from concourse.bass_utils import run_bass_kernel_spmd
from concourse.library_overlay import lower_extended_insts

F32 = mybir.dt.float32
BF16 = mybir.dt.bfloat16
I16 = mybir.dt.int16

NCORES = 8
NBLK = 4  # int16 gather-index blocks (N/NBLK must be < 32768)
R_TILES = 4  # dst tiles per gather range
GCAP = 4  # chunks (x128 idx) per dma_gather call
NQUEUES = 4  # SWDGE queues
ROW = 128  # bf16 elements per table row (256B: 64 features + 64 pad)

MAX_WAITS = 1  # this walrus build rejects >1 semaphore wait per instruction


def _split_excess_waits(nc, max_waits=MAX_WAITS):
    """Move excess sem-waits onto injected same-engine NOPs placed before the
    over-subscribed instruction (same-engine program order keeps semantics)."""
    uid = 0
    for f in nc.m.functions:
        for bb in f.blocks:
            il = bb.instructions
            new_il = []
            for inst in il:
                si = inst.sync_info
                waits = list(si.on_wait) if si and si.on_wait else []
                if len(waits) > max_waits:
                    excess, keep = waits[:-max_waits], waits[-max_waits:]
                    for j in range(0, len(excess), max_waits):
                        grp = excess[j : j + max_waits]
                        nop = mybir.InstNoOp(name=f"I-waitsplit-{uid}", ins=[], outs=[])
                        uid += 1
                        nop.engine = inst.engine
                        nop.sync_info = mybir.SyncInfo(on_wait=grp, on_update=[])
                        nc.register_instruction(nop, overwrite=True)
                        new_il.append(nop)
                    si.on_wait = keep
                new_il.append(inst)
            il[:] = new_il


def _finalize(nc):
    lower_extended_insts(nc)
    _split_excess_waits(nc)


# ---------------------------------------------------------------- L1: matmuls


def _build_l1(n_nodes, nfeat, nhid, ncls):
    shard = n_nodes // NCORES
    nc = bass.Bass()
    xT = nc.dram_tensor("xT", [nfeat, shard], F32, kind="ExternalInput")
    W1 = nc.dram_tensor("W1", [nfeat, nhid], F32, kind="ExternalInput")
    b1 = nc.dram_tensor("b1", [nhid, 1], F32, kind="ExternalInput")
    W2 = nc.dram_tensor("W2", [nhid, ncls], F32, kind="ExternalInput")
    table = nc.dram_tensor("table", [shard, ROW], BF16, kind="ExternalOutput")

    kchunks = nfeat // 128
    assert nfeat % 128 == 0 and nhid == 128
    NCHW = 512  # node columns per h chunk
    nch = (shard + NCHW - 1) // NCHW
    ntiles = (shard + 127) // 128

    with tile.TileContext(nc) as tc:
        with (
            tc.tile_pool(name="const", bufs=1) as constp,
            tc.tile_pool(name="xbuf", bufs=3) as xbuf,
            tc.tile_pool(name="hbuf", bufs=1) as hbuf,
            tc.tile_pool(name="obuf", bufs=4) as obuf,
            tc.tile_pool(name="psh", bufs=4, space="PSUM") as psh,
            tc.tile_pool(name="pss", bufs=4, space="PSUM") as pss,
        ):
            w1s = constp.tile([128, kchunks, nhid], F32)
            nc.sync.dma_start(
                out=w1s[:], in_=W1[:].rearrange("(k p) h -> p k h", p=128)
            )
            w2s = constp.tile([128, ncls], F32)
            nc.sync.dma_start(out=w2s[:], in_=W2[:])
            b1s = constp.tile([128, 1], F32)
            nc.sync.dma_start(out=b1s[:], in_=b1[:])

            hT = hbuf.tile([128, shard], F32)  # resident h.T (fp32)
            for j in range(nch):
                j0 = j * NCHW
                nsz = min(NCHW, shard - j0)
                xt = xbuf.tile([128, kchunks, NCHW], F32, tag="xt")
                nc.sync.dma_start(
                    out=xt[:, :, :nsz],
                    in_=xT[:, j0 : j0 + nsz].rearrange("(k p) n -> p k n", p=128),
                )
                ph = psh.tile([128, NCHW], F32)
                for k in range(kchunks):
                    nc.tensor.matmul(
                        ph[:, :nsz],
                        w1s[:, k, :],
                        xt[:, k, :nsz],
                        start=(k == 0),
                        stop=(k == kchunks - 1),
                    )
                nc.scalar.activation(
                    hT[:, j0 : j0 + nsz],
                    ph[:, :nsz],
                    mybir.ActivationFunctionType.Relu,
                    bias=b1s[:],
                    scale=1.0,
                )
            for t in range(ntiles):
                t0 = t * 128
                msz = min(128, shard - t0)
                ps = pss.tile([128, ncls], F32)
                nc.tensor.matmul(
                    ps[:msz, :], hT[:, t0 : t0 + msz], w2s[:], start=True, stop=True
                )
                ob = obuf.tile([128, ROW], BF16, tag="ob")
                nc.vector.memset(ob[:, ncls:], 0.0)
                nc.vector.tensor_copy(ob[:msz, :ncls], ps[:msz, :])
                nc.sync.dma_start(out=table[t0 : t0 + msz, :], in_=ob[:msz, :])

    _finalize(nc)
    return nc


# ------------------------------------------------- edge schedule (host side)


def _edge_schedule(src, dst, ew, n_nodes, shard):
    """Partition edges by destination shard, sort by (dst tile, src block),
    build the SPMD-common gather/compute schedule (max counts over cores) and
    each core's index/weight streams laid into that skeleton.

    Returns (schedule, percore, dims).
    """
    blk = n_nodes // NBLK
    ntiles = (shard + 127) // 128
    core_of = dst // shard

    percore_edges = []
    cnt_all = np.zeros((NCORES, ntiles, NBLK), np.int64)
    for c in range(NCORES):
        m = core_of == c
        s = src[m]
        d = dst[m] - c * shard
        w = ew[m]
        tl = d // 128
        bl = s // blk
        order = np.lexsort((bl, tl))
        s, d, w, tl, bl = s[order], d[order], w[order], tl[order], bl[order]
        cnt = np.zeros((ntiles, NBLK), np.int64)
        np.add.at(cnt, (tl, bl), 1)
        cnt_all[c] = cnt
        run_off = np.zeros(ntiles * NBLK + 1, np.int64)
        np.cumsum(cnt.reshape(-1), out=run_off[1:])
        percore_edges.append((s, d, w, cnt, run_off))

    # SPMD skeleton: chunks per (tile, block) = ceil(max-over-cores / 128)
    nr_tb = (cnt_all.max(axis=0) + 127) // 128
    for t in range(ntiles):
        if nr_tb[t].sum() == 0:
            nr_tb[t, 0] = 1  # keep every tile non-empty

    nranges = (ntiles + R_TILES - 1) // R_TILES
    schedule = []
    icol_off = 0
    chunk_off = 0
    gmax = 1
    run_loc = {}  # (t, b) -> (icol, chunkcol, nchunks)
    for r in range(nranges):
        tlo, thi = r * R_TILES, min(ntiles, r * R_TILES + R_TILES)
        gathers = []
        for b in range(NBLK):
            nch_rb = int(nr_tb[tlo:thi, b].sum())
            if nch_rb == 0:
                continue
            ncalls = (nch_rb + GCAP - 1) // GCAP
            nch_pad = ncalls * GCAP
            off = 0
            for t in range(tlo, thi):
                if nr_tb[t, b]:
                    run_loc[(t, b)] = (
                        icol_off + off * 8,
                        chunk_off + off,
                        int(nr_tb[t, b]),
                    )
                    off += int(nr_tb[t, b])
            gathers.append(
                dict(
                    b=b,
                    icol=icol_off,
                    chunk0=chunk_off,
                    nchunks=nch_rb,
                    nch_pad=nch_pad,
                    ncalls=ncalls,
                )
            )
            gmax = max(gmax, nch_pad)
            icol_off += nch_pad * 8  # 128 idx per chunk = 8 cols of 16
            chunk_off += nch_rb
        tiles = []
        for t in range(tlo, thi):
            msz = min(128, shard - t * 128)
            runs = []
            for g in gathers:
                b = g["b"]
                if nr_tb[t, b]:
                    icol, chcol, nchk = run_loc[(t, b)]
                    runs.append((b, chcol - g["chunk0"], nchk, chcol))
            tiles.append(dict(t=t, msz=msz, runs=runs))
        schedule.append(dict(gathers=gathers, tiles=tiles))

    icols = max(icol_off, 16)
    tch = max(chunk_off, 1)

    percore = []
    for c in range(NCORES):
        s, d, w, cnt, run_off = percore_edges[c]
        idx_flat = np.zeros(tch * 128, np.int16)
        dst_flat = np.zeros(tch * 128, np.float32)
        w_flat = np.zeros(tch * 128, np.float32)
        for (t, b), (icol, chcol, nchk) in run_loc.items():
            n_real = int(cnt[t, b])
            if n_real == 0:
                continue
            i0 = int(run_off[t * NBLK + b])
            o0 = chcol * 128
            idx_flat[o0 : o0 + n_real] = (s[i0 : i0 + n_real] - b * blk).astype(
                np.int16
            )
            dst_flat[o0 : o0 + n_real] = (d[i0 : i0 + n_real] - t * 128).astype(
                np.float32
            )
            w_flat[o0 : o0 + n_real] = w[i0 : i0 + n_real]
        # lay real-chunk idx into the padded call skeleton; pad chunks gather
        # row 0 (finite, weight-0) so every call is a uniform full 512 idx
        idx_cols = np.zeros((icols // 8, 128), np.int16)
        for rng_ in schedule:
            for g in rng_["gathers"]:
                nch, c0 = g["nchunks"], g["chunk0"]
                block = idx_flat[c0 * 128 : (c0 + nch) * 128]
                base = g["icol"] // 8
                idx_cols[base : base + nch] = block.reshape(nch, 128)
        idx16 = np.tile(idx_cols.reshape(-1, 16).T, (8, 1))  # [128, icols]
        dstw = dst_flat.reshape(tch, 128).T.copy()
        wmat = w_flat.reshape(tch, 128).T.copy()
        percore.append(
            dict(
                idx=np.ascontiguousarray(idx16),
                dstw=np.ascontiguousarray(dstw),
                wmat=np.ascontiguousarray(wmat),
            )
        )

    fp = hash((nr_tb.tobytes(), shard, n_nodes))
    dims = dict(icols=icols, tch=tch, gmax=gmax, fingerprint=fp)
    return schedule, percore, dims


# ---------------------------------------------------------------- L2: edges


def _build_l2(n_nodes, ncls, shard, schedule, dims):
    blk = n_nodes // NBLK
    icols, tch, gmax = dims["icols"], dims["tch"], dims["gmax"]
    nc = bass.Bass(num_swdge_queues=NQUEUES)
    table = nc.dram_tensor("table", [n_nodes, ROW], BF16, kind="ExternalInput")
    idxs = nc.dram_tensor("idxs", [128, icols], I16, kind="ExternalInput")
    dstw = nc.dram_tensor("dstw", [128, tch], BF16, kind="ExternalInput")
    wmat = nc.dram_tensor("wmat", [128, tch], BF16, kind="ExternalInput")
    b2t = nc.dram_tensor("b2t", [ncls, 1], F32, kind="ExternalInput")
    aggT = nc.dram_tensor("aggT", [ncls, shard], F32, kind="ExternalOutput")

    iota_np = np.tile(np.arange(128, dtype=np.float32), (128, 1))
    iota_t = nc.inline_tensor(iota_np, "iota")

    from contextlib import ExitStack

    with tile.TileContext(nc) as tc, ExitStack() as es:
        nidx_reg = es.enter_context(nc.gpsimd.register("nidx_reg"))
        with (
            tc.tile_pool(name="const", bufs=1) as constp,
            tc.tile_pool(name="idxp", bufs=2) as idxp,
            tc.tile_pool(name="gp", bufs=2) as gp,
            tc.tile_pool(name="ohp", bufs=4) as ohp,
            tc.tile_pool(name="evp", bufs=4) as evp,
            tc.tile_pool(name="psp", bufs=6, space="PSUM") as psp,
        ):
            nc.gpsimd.load_library(library_config.mlp)
            iota_f32 = constp.tile([128, 128], F32)
            nc.sync.dma_start(out=iota_f32[:], in_=iota_t[:])
            iota_s = constp.tile([128, 128], BF16)
            nc.vector.tensor_copy(iota_s[:], iota_f32[:])
            b2s = constp.tile([ncls, 1], F32)
            nc.sync.dma_start(out=b2s[:], in_=b2t[:])
            dstw_s = constp.tile([128, tch], BF16)
            nc.sync.dma_start(out=dstw_s[:], in_=dstw[:])
            wmat_s = constp.tile([128, tch], BF16)
            nc.sync.dma_start(out=wmat_s[:], in_=wmat[:])

            nc.gpsimd.reg_mov(nidx_reg, GCAP * 128)
            qn = 0
            for rng in schedule:
                gathers = rng["gathers"]
                gbufs = {}
                if gathers:
                    icol0 = gathers[0]["icol"]
                    icoln = gathers[-1]["icol"] + gathers[-1]["nch_pad"] * 8
                    ib = idxp.tile([128, icoln - icol0], I16, tag="idx")
                    nc.sync.dma_start(out=ib[:], in_=idxs[:, icol0:icoln])
                for g in gathers:
                    b = g["b"]
                    gb = gp.tile([128, g["nch_pad"], ROW], BF16, tag=f"g{b}")
                    gbufs[b] = g
                    g["tile"] = gb
                    nch = g["nchunks"]
                    for k in range(g["ncalls"]):
                        c_lo = k * GCAP
                        ic = g["icol"] - icol0 + c_lo * 8
                        nc.gpsimd.dma_gather(
                            gb[:, c_lo : c_lo + GCAP, :],
                            table[b * blk : (b + 1) * blk, :],
                            ib[:, ic : ic + GCAP * 8],
                            GCAP * 128,
                            nidx_reg,
                            ROW,
                            single_packet=True,
                            queue_num=qn,
                        )
                        qn = (qn + 1) % NQUEUES
                    # scale messages by edge weight (batched, bf16 2x)
                    c0 = g["chunk0"]
                    nc.vector.tensor_tensor(
                        gb[:, :nch, :ncls],
                        gb[:, :nch, :ncls],
                        wmat_s[:, c0 : c0 + nch]
                        .unsqueeze(2)
                        .to_broadcast((128, nch, ncls)),
                        mybir.AluOpType.mult,
                    )
                for tt in rng["tiles"]:
                    t, msz, runs = tt["t"], tt["msz"], tt["runs"]
                    ps = psp.tile([ncls, 128], F32, tag="ps")
                    nchunks_t = sum(nr for (_, _, nr, _) in runs)
                    ci = 0
                    for b, c0, nr, chcol in runs:
                        g = gbufs[b]
                        gb = g["tile"]
                        oh = ohp.tile([128, nr, 128], BF16, tag="oh")
                        nc.vector.tensor_tensor(
                            oh[:],
                            dstw_s[:, chcol : chcol + nr]
                            .unsqueeze(2)
                            .to_broadcast((128, nr, 128)),
                            iota_s[:].unsqueeze(1).to_broadcast((128, nr, 128)),
                            mybir.AluOpType.is_equal,
                        )
                        for j in range(nr):
                            nc.tensor.matmul(
                                ps[:, :msz],
                                gb[:, c0 + j, :ncls],
                                oh[:, j, :msz],
                                start=(ci == 0),
                                stop=(ci == nchunks_t - 1),
                            )
                            ci += 1
                    ev = evp.tile([ncls, 128], F32, tag="ev")
                    nc.vector.tensor_scalar_add(ev[:, :msz], ps[:, :msz], b2s[:])
                    nc.sync.dma_start(
                        out=aggT[:, t * 128 : t * 128 + msz], in_=ev[:, :msz]
                    )

    _finalize(nc)
    return nc


# ------------------------------------------------------------------- driver

_CACHE = {}
LAST_TIMES = {}


def _timed_run(name, nc, in_maps, core_ids):
    import time as _time

    t0 = _time.time()
    res = run_bass_kernel_spmd(nc, in_maps, core_ids)
    LAST_TIMES[name] = _time.time() - t0
    return res


def kernel(x, W1, b1, W2, b2, edge_index, edge_weight):
    x = np.asarray(x, np.float32)
    W1 = np.asarray(W1, np.float32)
    b1 = np.asarray(b1, np.float32)
    W2 = np.asarray(W2, np.float32)
    b2 = np.asarray(b2, np.float32)
    edge_index = np.asarray(edge_index)
    edge_weight = np.asarray(edge_weight, np.float32)

    n_nodes, nfeat = x.shape
    ncls = W2.shape[1]
    shard = n_nodes // NCORES
    core_ids = list(range(NCORES))

    # ---- L1: support table ----
    key1 = ("l1", n_nodes, nfeat, W1.shape[1], ncls)
    if key1 not in _CACHE:
        _CACHE[key1] = _build_l1(n_nodes, nfeat, W1.shape[1], ncls)
    nc1 = _CACHE[key1]

    xT = np.ascontiguousarray(x.T)
    in_maps1 = [
        {
            "xT": np.ascontiguousarray(xT[:, c * shard : (c + 1) * shard]),
            "W1": W1,
            "b1": np.ascontiguousarray(b1.reshape(-1, 1)),
            "W2": W2,
        }
        for c in core_ids
    ]
    res1 = _timed_run("l1", nc1, in_maps1, core_ids)
    table = np.ascontiguousarray(
        np.concatenate([res1.results[c]["table"] for c in core_ids], axis=0)
    )

    # ---- host edge preprocessing ----
    src = edge_index[0].astype(np.int64)
    dst = edge_index[1].astype(np.int64)
    ekey = ("sched", n_nodes, shard, edge_index.shape[1])
    if ekey in _CACHE and _CACHE[ekey][0] is not None:
        fph, schedule, percore, dims = _CACHE[ekey]
        if fph != hash(edge_index.tobytes()):
            schedule = None
    else:
        schedule = None
    if schedule is None:
        schedule, percore, dims = _edge_schedule(
            src, dst, edge_weight, n_nodes, shard
        )
        _CACHE[ekey] = (hash(edge_index.tobytes()), schedule, percore, dims)

    key2 = ("l2", n_nodes, ncls, shard, dims["fingerprint"])
    if key2 not in _CACHE:
        _CACHE[key2] = _build_l2(n_nodes, ncls, shard, schedule, dims)
    nc2 = _CACHE[key2]

    import ml_dtypes

    b2c = np.ascontiguousarray(b2.reshape(-1, 1))
    in_maps2 = [
        {
            "table": table,
            "idxs": percore[c]["idx"],
            "dstw": percore[c]["dstw"].astype(ml_dtypes.bfloat16),
            "wmat": percore[c]["wmat"].astype(ml_dtypes.bfloat16),
            "b2t": b2c,
        }
        for c in core_ids
    ]
    res2 = _timed_run("l2", nc2, in_maps2, core_ids)
    out = np.concatenate(
        [np.ascontiguousarray(res2.results[c]["aggT"].T.astype(np.float32)) for c in core_ids],
        axis=0,
    )
    return out


# revision 17
# speedup vs baseline: 3.0287x; 1.2607x over previous
"""GCN layer (x@W1 -> relu -> @W2 -> weighted scatter-add over edges) on 8 TRN2 cores.

Strategy (two launches, 8-way SPMD):
  L1: row-shard x across cores; each core computes its shard of
      support = relu(x@W1+b1)@W2 via TensorE (fp32 accumulate), writes its
      [N/8, 128] bf16 table shard (64 features + 64-byte pad -> 256B rows,
      the dma_gather minimum row size). Host concatenates shards.
  L2: edges partitioned by destination shard (core = dst // (N/8)).
      Per core: dma_gather (GPSIMD SWDGE, 4 queues, 512-idx calls, deep
      buffering) fetches 256B table rows per edge; DVE scales messages by
      edge weight (batched tensor_tensor) and builds 0/1 one-hot matrices
      (batched is_equal against an iota tile); TensorE accumulates
      agg.T[64, dst_tile] += msg.T @ onehot in PSUM; bias b2 added at PSUM
      evacuation. Host transposes/concats shards.

All floating-point math happens on device; the host only shards, sorts edge
indices, and concatenates outputs.
"""

import sys

if "/opt/trn_rl_repo" not in sys.path:
    sys.path.insert(0, "/opt/trn_rl_repo")

import numpy as np

import concourse.bass as bass
import concourse.tile as tile
from concourse import library_config, mybir
from concourse.bass_utils import run_bass_kernel_spmd
from concourse.library_overlay import lower_extended_insts

F32 = mybir.dt.float32
BF16 = mybir.dt.bfloat16
I16 = mybir.dt.int16

NCORES = 8
NBLK = 4  # int16 gather-index blocks (N/NBLK must be < 32768)
R_TILES = 4  # dst tiles per gather range
GCAP = 8  # chunks (x128 idx) per dma_gather call
NQUEUES = 4  # SWDGE queues
DMA_SCRATCH = 65536  # SWDGE descriptor-ring carveout (deeper in-flight window)
ACT_FRAC = 0.7  # fraction of each group's message-scales run on ACT (rest DVE)
ROW = 128  # bf16 elements per table row (256B: 64 features + 64 pad)

MAX_WAITS = 1  # this walrus build rejects >1 semaphore wait per instruction


def _split_excess_waits(nc, max_waits=MAX_WAITS):
    """Move excess sem-waits onto injected same-engine NOPs placed before the
    over-subscribed instruction (same-engine program order keeps semantics)."""
    uid = 0
    for f in nc.m.functions:
        for bb in f.blocks:
            il = bb.instructions
            new_il = []
            for inst in il:
                si = inst.sync_info
                waits = list(si.on_wait) if si and si.on_wait else []
                if len(waits) > max_waits:
                    excess, keep = waits[:-max_waits], waits[-max_waits:]
                    for j in range(0, len(excess), max_waits):
                        grp = excess[j : j + max_waits]
                        nop = mybir.InstNoOp(name=f"I-waitsplit-{uid}", ins=[], outs=[])
                        uid += 1
                        nop.engine = inst.engine
                        nop.sync_info = mybir.SyncInfo(on_wait=grp, on_update=[])
                        nc.register_instruction(nop, overwrite=True)
                        new_il.append(nop)
                    si.on_wait = keep
                new_il.append(inst)
            il[:] = new_il


def _finalize(nc):
    lower_extended_insts(nc)
    _split_excess_waits(nc)


# ---------------------------------------------------------------- L1: matmuls


def _build_l1(n_nodes, nfeat, nhid, ncls):
    shard = n_nodes // NCORES
    nc = bass.Bass()
    xT = nc.dram_tensor("xT", [nfeat, shard], F32, kind="ExternalInput")
    W1 = nc.dram_tensor("W1", [nfeat, nhid], F32, kind="ExternalInput")
    b1 = nc.dram_tensor("b1", [nhid, 1], F32, kind="ExternalInput")
    W2 = nc.dram_tensor("W2", [nhid, ncls], F32, kind="ExternalInput")
    table = nc.dram_tensor("table", [shard, ROW], BF16, kind="ExternalOutput")

    kchunks = nfeat // 128
    assert nfeat % 128 == 0 and nhid == 128
    NCHW = 512  # node columns per h chunk
    nch = (shard + NCHW - 1) // NCHW
    ntiles = (shard + 127) // 128

    with tile.TileContext(nc) as tc:
        with (
            tc.tile_pool(name="const", bufs=1) as constp,
            tc.tile_pool(name="xbuf", bufs=3) as xbuf,
            tc.tile_pool(name="hbuf", bufs=1) as hbuf,
            tc.tile_pool(name="obuf", bufs=4) as obuf,
            tc.tile_pool(name="psh", bufs=4, space="PSUM") as psh,
            tc.tile_pool(name="pss", bufs=4, space="PSUM") as pss,
        ):
            w1s = constp.tile([128, kchunks, nhid], F32)
            nc.sync.dma_start(
                out=w1s[:], in_=W1[:].rearrange("(k p) h -> p k h", p=128)
            )
            w2s = constp.tile([128, ncls], F32)
            nc.sync.dma_start(out=w2s[:], in_=W2[:])
            b1s = constp.tile([128, 1], F32)
            nc.sync.dma_start(out=b1s[:], in_=b1[:])

            hT = hbuf.tile([128, shard], F32)  # resident h.T (fp32)
            for j in range(nch):
                j0 = j * NCHW
                nsz = min(NCHW, shard - j0)
                xt = xbuf.tile([128, kchunks, NCHW], F32, tag="xt")
                nc.sync.dma_start(
                    out=xt[:, :, :nsz],
                    in_=xT[:, j0 : j0 + nsz].rearrange("(k p) n -> p k n", p=128),
                )
                ph = psh.tile([128, NCHW], F32)
                for k in range(kchunks):
                    nc.tensor.matmul(
                        ph[:, :nsz],
                        w1s[:, k, :],
                        xt[:, k, :nsz],
                        start=(k == 0),
                        stop=(k == kchunks - 1),
                    )
                nc.scalar.activation(
                    hT[:, j0 : j0 + nsz],
                    ph[:, :nsz],
                    mybir.ActivationFunctionType.Relu,
                    bias=b1s[:],
                    scale=1.0,
                )
            for t in range(ntiles):
                t0 = t * 128
                msz = min(128, shard - t0)
                ps = pss.tile([128, ncls], F32)
                nc.tensor.matmul(
                    ps[:msz, :], hT[:, t0 : t0 + msz], w2s[:], start=True, stop=True
                )
                ob = obuf.tile([128, ROW], BF16, tag="ob")
                nc.vector.memset(ob[:, ncls:], 0.0)
                nc.vector.tensor_copy(ob[:msz, :ncls], ps[:msz, :])
                nc.sync.dma_start(out=table[t0 : t0 + msz, :], in_=ob[:msz, :])

    _finalize(nc)
    return nc


# ------------------------------------------------- edge schedule (host side)


def _edge_schedule(src, dst, ew, n_nodes, shard):
    """Partition edges by destination shard, sort by (dst tile, src block),
    build the SPMD-common gather/compute schedule (max counts over cores) and
    each core's index/weight streams laid into that skeleton.

    Returns (schedule, percore, dims).
    """
    blk = n_nodes // NBLK
    ntiles = (shard + 127) // 128
    core_of = dst // shard

    percore_edges = []
    cnt_all = np.zeros((NCORES, ntiles, NBLK), np.int64)
    for c in range(NCORES):
        m = core_of == c
        s = src[m]
        d = dst[m] - c * shard
        w = ew[m]
        tl = d // 128
        bl = s // blk
        order = np.lexsort((bl, tl))
        s, d, w, tl, bl = s[order], d[order], w[order], tl[order], bl[order]
        cnt = np.zeros((ntiles, NBLK), np.int64)
        np.add.at(cnt, (tl, bl), 1)
        cnt_all[c] = cnt
        run_off = np.zeros(ntiles * NBLK + 1, np.int64)
        np.cumsum(cnt.reshape(-1), out=run_off[1:])
        percore_edges.append((s, d, w, cnt, run_off))

    # SPMD skeleton: chunks per (tile, block) = ceil(max-over-cores / 128)
    nr_tb = (cnt_all.max(axis=0) + 127) // 128
    for t in range(ntiles):
        if nr_tb[t].sum() == 0:
            nr_tb[t, 0] = 1  # keep every tile non-empty

    nranges = (ntiles + R_TILES - 1) // R_TILES
    schedule = []
    icol_off = 0
    chunk_off = 0
    gmax = 1
    run_loc = {}  # (t, b) -> (icol, chunkcol, nchunks)
    for r in range(nranges):
        tlo, thi = r * R_TILES, min(ntiles, r * R_TILES + R_TILES)
        gathers = []
        for b in range(NBLK):
            nch_rb = int(nr_tb[tlo:thi, b].sum())
            if nch_rb == 0:
                continue
            ncalls = (nch_rb + GCAP - 1) // GCAP
            nch_pad = ncalls * GCAP
            off = 0
            for t in range(tlo, thi):
                if nr_tb[t, b]:
                    run_loc[(t, b)] = (
                        icol_off + off * 8,
                        chunk_off + off,
                        int(nr_tb[t, b]),
                    )
                    off += int(nr_tb[t, b])
            gathers.append(
                dict(
                    b=b,
                    icol=icol_off,
                    chunk0=chunk_off,
                    nchunks=nch_rb,
                    nch_pad=nch_pad,
                    ncalls=ncalls,
                )
            )
            gmax = max(gmax, nch_pad)
            icol_off += nch_pad * 8  # 128 idx per chunk = 8 cols of 16
            chunk_off += nch_rb
        tiles = []
        for t in range(tlo, thi):
            msz = min(128, shard - t * 128)
            runs = []
            for g in gathers:
                b = g["b"]
                if nr_tb[t, b]:
                    icol, chcol, nchk = run_loc[(t, b)]
                    runs.append((b, chcol - g["chunk0"], nchk, chcol))
            tiles.append(dict(t=t, msz=msz, runs=runs))
        schedule.append(dict(gathers=gathers, tiles=tiles))

    icols = max(icol_off, 16)
    tch = max(chunk_off, 1)

    percore = []
    for c in range(NCORES):
        s, d, w, cnt, run_off = percore_edges[c]
        idx_flat = np.zeros(tch * 128, np.int16)
        dst_flat = np.zeros(tch * 128, np.float32)
        w_flat = np.zeros(tch * 128, np.float32)
        for (t, b), (icol, chcol, nchk) in run_loc.items():
            n_real = int(cnt[t, b])
            if n_real == 0:
                continue
            i0 = int(run_off[t * NBLK + b])
            o0 = chcol * 128
            idx_flat[o0 : o0 + n_real] = (s[i0 : i0 + n_real] - b * blk).astype(
                np.int16
            )
            dst_flat[o0 : o0 + n_real] = (d[i0 : i0 + n_real] - t * 128).astype(
                np.float32
            )
            w_flat[o0 : o0 + n_real] = w[i0 : i0 + n_real]
        # lay real-chunk idx into the padded call skeleton; pad chunks gather
        # row 0 (finite, weight-0) so every call is a uniform full 512 idx
        idx_cols = np.zeros((icols // 8, 128), np.int16)
        for rng_ in schedule:
            for g in rng_["gathers"]:
                nch, c0 = g["nchunks"], g["chunk0"]
                block = idx_flat[c0 * 128 : (c0 + nch) * 128]
                base = g["icol"] // 8
                idx_cols[base : base + nch] = block.reshape(nch, 128)
        idx16 = np.tile(idx_cols.reshape(-1, 16).T, (8, 1))  # [128, icols]
        dstw = dst_flat.reshape(tch, 128).T.copy()
        wmat = w_flat.reshape(tch, 128).T.copy()
        percore.append(
            dict(
                idx=np.ascontiguousarray(idx16),
                dstw=np.ascontiguousarray(dstw),
                wmat=np.ascontiguousarray(wmat),
            )
        )

    fp = hash((nr_tb.tobytes(), shard, n_nodes))
    dims = dict(icols=icols, tch=tch, gmax=gmax, fingerprint=fp)
    return schedule, percore, dims


# ---------------------------------------------------------------- L2: edges


def _build_l2(n_nodes, ncls, shard, schedule, dims):
    blk = n_nodes // NBLK
    icols, tch, gmax = dims["icols"], dims["tch"], dims["gmax"]
    nc = bass.Bass(num_swdge_queues=NQUEUES, dynamic_dma_scratch_size=DMA_SCRATCH)
    table = nc.dram_tensor("table", [n_nodes, ROW], BF16, kind="ExternalInput")
    idxs = nc.dram_tensor("idxs", [128, icols], I16, kind="ExternalInput")
    dstw = nc.dram_tensor("dstw", [128, tch], BF16, kind="ExternalInput")
    wmat = nc.dram_tensor("wmat", [128, tch], BF16, kind="ExternalInput")
    b2t = nc.dram_tensor("b2t", [ncls, 1], F32, kind="ExternalInput")
    aggT = nc.dram_tensor("aggT", [ncls, shard], F32, kind="ExternalOutput")

    iota_np = np.tile(np.arange(128, dtype=np.float32), (128, 1))
    iota_t = nc.inline_tensor(iota_np, "iota")

    from contextlib import ExitStack

    with tile.TileContext(nc) as tc, ExitStack() as es:
        nidx_reg = es.enter_context(nc.gpsimd.register("nidx_reg"))
        with (
            tc.tile_pool(name="const", bufs=1) as constp,
            tc.tile_pool(name="idxp", bufs=2) as idxp,
            tc.tile_pool(name="gp", bufs=2) as gp,
            tc.tile_pool(name="ohp", bufs=4) as ohp,
            tc.tile_pool(name="evp", bufs=4) as evp,
            tc.tile_pool(name="psp", bufs=6, space="PSUM") as psp,
        ):
            nc.gpsimd.load_library(library_config.mlp)
            iota_f32 = constp.tile([128, 128], F32)
            nc.sync.dma_start(out=iota_f32[:], in_=iota_t[:])
            iota_s = constp.tile([128, 128], BF16)
            nc.vector.tensor_copy(iota_s[:], iota_f32[:])
            b2s = constp.tile([ncls, 1], F32)
            nc.sync.dma_start(out=b2s[:], in_=b2t[:])
            dstw_s = constp.tile([128, tch], BF16)
            nc.sync.dma_start(out=dstw_s[:], in_=dstw[:])
            wmat_s = constp.tile([128, tch], BF16)
            nc.sync.dma_start(out=wmat_s[:], in_=wmat[:])
            wmat_f = constp.tile([128, tch], F32)
            nc.vector.tensor_copy(wmat_f[:], wmat_s[:])

            nc.gpsimd.reg_mov(nidx_reg, GCAP * 128)
            qn = 0
            gcount = 0
            for rng in schedule:
                gathers = rng["gathers"]
                gbufs = {}
                if gathers:
                    icol0 = gathers[0]["icol"]
                    icoln = gathers[-1]["icol"] + gathers[-1]["nch_pad"] * 8
                    ib = idxp.tile([128, icoln - icol0], I16, tag="idx")
                    nc.sync.dma_start(out=ib[:], in_=idxs[:, icol0:icoln])
                for g in gathers:
                    b = g["b"]
                    gb = gp.tile([128, g["nch_pad"], ROW], BF16, tag=f"g{b}")
                    gbufs[b] = g
                    g["tile"] = gb
                    nch = g["nchunks"]
                    for k in range(g["ncalls"]):
                        c_lo = k * GCAP
                        ic = g["icol"] - icol0 + c_lo * 8
                        nc.gpsimd.dma_gather(
                            gb[:, c_lo : c_lo + GCAP, :],
                            table[b * blk : (b + 1) * blk, :],
                            ib[:, ic : ic + GCAP * 8],
                            GCAP * 128,
                            nidx_reg,
                            ROW,
                            single_packet=True,
                            queue_num=qn,
                        )
                        qn = (qn + 1) % NQUEUES
                    # scale messages by edge weight, split between the idle
                    # ACT engine (per-chunk Copy-with-scale) and DVE (batched)
                    c0g = g["chunk0"]
                    nact = int(round(nch * ACT_FRAC))
                    for j in range(nact):
                        nc.scalar.activation(
                            gb[:, j, :ncls],
                            gb[:, j, :ncls],
                            mybir.ActivationFunctionType.Copy,
                            bias=0.0,
                            scale=wmat_f[:, c0g + j : c0g + j + 1],
                        )
                    if nact < nch:
                        nc.vector.tensor_tensor(
                            gb[:, nact:nch, :ncls],
                            gb[:, nact:nch, :ncls],
                            wmat_s[:, c0g + nact : c0g + nch]
                            .unsqueeze(2)
                            .to_broadcast((128, nch - nact, ncls)),
                            mybir.AluOpType.mult,
                        )
                    gcount += 1
                    # one batched 0/1 one-hot build per gather group (its
                    # chunk columns are contiguous across the range's tiles)
                    oh = ohp.tile([128, nch, 128], BF16, tag="oh")
                    g["oh"] = oh
                    nc.vector.tensor_tensor(
                        oh[:],
                        dstw_s[:, c0g : c0g + nch]
                        .unsqueeze(2)
                        .to_broadcast((128, nch, 128)),
                        iota_s[:].unsqueeze(1).to_broadcast((128, nch, 128)),
                        mybir.AluOpType.is_equal,
                    )
                for tt in rng["tiles"]:
                    t, msz, runs = tt["t"], tt["msz"], tt["runs"]
                    ps = psp.tile([ncls, 128], F32, tag="ps")
                    nchunks_t = sum(nr for (_, _, nr, _) in runs)
                    ci = 0
                    for b, c0, nr, chcol in runs:
                        g = gbufs[b]
                        gb = g["tile"]
                        oh = g["oh"]
                        for j in range(nr):
                            nc.tensor.matmul(
                                ps[:, :msz],
                                gb[:, c0 + j, :ncls],
                                oh[:, c0 + j, :msz],
                                start=(ci == 0),
                                stop=(ci == nchunks_t - 1),
                            )
                            ci += 1
                    ev = evp.tile([ncls, 128], F32, tag="ev")
                    nc.vector.tensor_scalar_add(ev[:, :msz], ps[:, :msz], b2s[:])
                    nc.sync.dma_start(
                        out=aggT[:, t * 128 : t * 128 + msz], in_=ev[:, :msz]
                    )

    _finalize(nc)
    return nc


# ------------------------------------------------------------------- driver

_CACHE = {}
LAST_TIMES = {}


def _timed_run(name, nc, in_maps, core_ids):
    import time as _time

    t0 = _time.time()
    res = run_bass_kernel_spmd(nc, in_maps, core_ids)
    LAST_TIMES[name] = _time.time() - t0
    return res


def kernel(x, W1, b1, W2, b2, edge_index, edge_weight):
    x = np.asarray(x, np.float32)
    W1 = np.asarray(W1, np.float32)
    b1 = np.asarray(b1, np.float32)
    W2 = np.asarray(W2, np.float32)
    b2 = np.asarray(b2, np.float32)
    edge_index = np.asarray(edge_index)
    edge_weight = np.asarray(edge_weight, np.float32)

    n_nodes, nfeat = x.shape
    ncls = W2.shape[1]
    shard = n_nodes // NCORES
    core_ids = list(range(NCORES))

    # ---- L1: support table ----
    key1 = ("l1", n_nodes, nfeat, W1.shape[1], ncls)
    if key1 not in _CACHE:
        _CACHE[key1] = _build_l1(n_nodes, nfeat, W1.shape[1], ncls)
    nc1 = _CACHE[key1]

    xT = np.ascontiguousarray(x.T)
    in_maps1 = [
        {
            "xT": np.ascontiguousarray(xT[:, c * shard : (c + 1) * shard]),
            "W1": W1,
            "b1": np.ascontiguousarray(b1.reshape(-1, 1)),
            "W2": W2,
        }
        for c in core_ids
    ]
    res1 = _timed_run("l1", nc1, in_maps1, core_ids)
    table = np.ascontiguousarray(
        np.concatenate([res1.results[c]["table"] for c in core_ids], axis=0)
    )

    # ---- host edge preprocessing ----
    src = edge_index[0].astype(np.int64)
    dst = edge_index[1].astype(np.int64)
    ekey = ("sched", n_nodes, shard, edge_index.shape[1])
    if ekey in _CACHE and _CACHE[ekey][0] is not None:
        fph, schedule, percore, dims = _CACHE[ekey]
        if fph != hash(edge_index.tobytes()):
            schedule = None
    else:
        schedule = None
    if schedule is None:
        schedule, percore, dims = _edge_schedule(
            src, dst, edge_weight, n_nodes, shard
        )
        _CACHE[ekey] = (hash(edge_index.tobytes()), schedule, percore, dims)

    key2 = ("l2", n_nodes, ncls, shard, dims["fingerprint"])
    if key2 not in _CACHE:
        _CACHE[key2] = _build_l2(n_nodes, ncls, shard, schedule, dims)
    nc2 = _CACHE[key2]

    import ml_dtypes

    b2c = np.ascontiguousarray(b2.reshape(-1, 1))
    in_maps2 = [
        {
            "table": table,
            "idxs": percore[c]["idx"],
            "dstw": percore[c]["dstw"].astype(ml_dtypes.bfloat16),
            "wmat": percore[c]["wmat"].astype(ml_dtypes.bfloat16),
            "b2t": b2c,
        }
        for c in core_ids
    ]
    res2 = _timed_run("l2", nc2, in_maps2, core_ids)
    out = np.concatenate(
        [np.ascontiguousarray(res2.results[c]["aggT"].T.astype(np.float32)) for c in core_ids],
        axis=0,
    )
    return out


# revision 19
# speedup vs baseline: 11.9464x; 3.9444x over previous
"""GCN layer (x@W1 -> relu -> @W2 -> weighted scatter-add over edges) on 8 TRN2 cores.

Strategy (two launches, 8-way SPMD):
  L1: row-shard x across cores; each core computes its shard of
      support = relu(x@W1+b1)@W2 via TensorE (fp32 accumulate), writes its
      [N/8, 128] bf16 table shard (64 features + 64-byte pad -> 256B rows,
      the dma_gather minimum row size). Host concatenates shards.
  L2: edges partitioned by destination shard (core = dst // (N/8)).
      Per core: dma_gather (GPSIMD SWDGE, 4 queues, 512-idx calls, deep
      buffering) fetches 256B table rows per edge; DVE scales messages by
      edge weight (batched tensor_tensor) and builds 0/1 one-hot matrices
      (batched is_equal against an iota tile); TensorE accumulates
      agg.T[64, dst_tile] += msg.T @ onehot in PSUM; bias b2 added at PSUM
      evacuation. Host transposes/concats shards.

All floating-point math happens on device; the host only shards, sorts edge
indices, and concatenates outputs.
"""

import sys

if "/opt/trn_rl_repo" not in sys.path:
    sys.path.insert(0, "/opt/trn_rl_repo")

import numpy as np

import concourse.bass as bass
import concourse.tile as tile
from concourse import library_config, mybir
from concourse.bass_utils import run_bass_kernel_spmd
from concourse.library_overlay import lower_extended_insts

F32 = mybir.dt.float32
BF16 = mybir.dt.bfloat16
I16 = mybir.dt.int16

NCORES = 8
NBLK = 4  # int16 gather-index blocks (N/NBLK must be < 32768)
R_TILES = 4  # dst tiles per gather range
GCAP = 4  # chunks (x128 idx) per dma_gather call
NQUEUES = 4  # SWDGE queues
DMA_SCRATCH = 16384  # SWDGE descriptor-ring carveout
ACT_FRAC = 0.0  # fraction of each group's message-scales run on ACT (rest DVE)
ROW = 128  # bf16 elements per table row (256B: 64 features + 64 pad)

MAX_WAITS = 1  # this walrus build rejects >1 semaphore wait per instruction


def _split_excess_waits(nc, max_waits=MAX_WAITS):
    """Move excess sem-waits onto injected same-engine NOPs placed before the
    over-subscribed instruction (same-engine program order keeps semantics)."""
    uid = 0
    for f in nc.m.functions:
        for bb in f.blocks:
            il = bb.instructions
            new_il = []
            for inst in il:
                si = inst.sync_info
                waits = list(si.on_wait) if si and si.on_wait else []
                if len(waits) > max_waits:
                    excess, keep = waits[:-max_waits], waits[-max_waits:]
                    for j in range(0, len(excess), max_waits):
                        grp = excess[j : j + max_waits]
                        nop = mybir.InstNoOp(name=f"I-waitsplit-{uid}", ins=[], outs=[])
                        uid += 1
                        nop.engine = inst.engine
                        nop.sync_info = mybir.SyncInfo(on_wait=grp, on_update=[])
                        nc.register_instruction(nop, overwrite=True)
                        new_il.append(nop)
                    si.on_wait = keep
                new_il.append(inst)
            il[:] = new_il


def _finalize(nc):
    lower_extended_insts(nc)
    _split_excess_waits(nc)


# ---------------------------------------------------------------- L1: matmuls


def _build_l1(n_nodes, nfeat, nhid, ncls):
    shard = n_nodes // NCORES
    nc = bass.Bass()
    xT = nc.dram_tensor("xT", [nfeat, shard], F32, kind="ExternalInput")
    W1 = nc.dram_tensor("W1", [nfeat, nhid], F32, kind="ExternalInput")
    b1 = nc.dram_tensor("b1", [nhid, 1], F32, kind="ExternalInput")
    W2 = nc.dram_tensor("W2", [nhid, ncls], F32, kind="ExternalInput")
    table = nc.dram_tensor("table", [shard, ROW], BF16, kind="ExternalOutput")

    kchunks = nfeat // 128
    assert nfeat % 128 == 0 and nhid == 128
    NCHW = 512  # node columns per h chunk
    nch = (shard + NCHW - 1) // NCHW
    ntiles = (shard + 127) // 128

    with tile.TileContext(nc) as tc:
        with (
            tc.tile_pool(name="const", bufs=1) as constp,
            tc.tile_pool(name="xbuf", bufs=3) as xbuf,
            tc.tile_pool(name="hbuf", bufs=1) as hbuf,
            tc.tile_pool(name="obuf", bufs=4) as obuf,
            tc.tile_pool(name="psh", bufs=4, space="PSUM") as psh,
            tc.tile_pool(name="pss", bufs=4, space="PSUM") as pss,
        ):
            w1s = constp.tile([128, kchunks, nhid], F32)
            nc.sync.dma_start(
                out=w1s[:], in_=W1[:].rearrange("(k p) h -> p k h", p=128)
            )
            w2s = constp.tile([128, ncls], F32)
            nc.sync.dma_start(out=w2s[:], in_=W2[:])
            b1s = constp.tile([128, 1], F32)
            nc.sync.dma_start(out=b1s[:], in_=b1[:])

            hT = hbuf.tile([128, shard], F32)  # resident h.T (fp32)
            for j in range(nch):
                j0 = j * NCHW
                nsz = min(NCHW, shard - j0)
                xt = xbuf.tile([128, kchunks, NCHW], F32, tag="xt")
                nc.sync.dma_start(
                    out=xt[:, :, :nsz],
                    in_=xT[:, j0 : j0 + nsz].rearrange("(k p) n -> p k n", p=128),
                )
                ph = psh.tile([128, NCHW], F32)
                for k in range(kchunks):
                    nc.tensor.matmul(
                        ph[:, :nsz],
                        w1s[:, k, :],
                        xt[:, k, :nsz],
                        start=(k == 0),
                        stop=(k == kchunks - 1),
                    )
                nc.scalar.activation(
                    hT[:, j0 : j0 + nsz],
                    ph[:, :nsz],
                    mybir.ActivationFunctionType.Relu,
                    bias=b1s[:],
                    scale=1.0,
                )
            for t in range(ntiles):
                t0 = t * 128
                msz = min(128, shard - t0)
                ps = pss.tile([128, ncls], F32)
                nc.tensor.matmul(
                    ps[:msz, :], hT[:, t0 : t0 + msz], w2s[:], start=True, stop=True
                )
                ob = obuf.tile([128, ROW], BF16, tag="ob")
                nc.vector.memset(ob[:, ncls:], 0.0)
                nc.vector.tensor_copy(ob[:msz, :ncls], ps[:msz, :])
                nc.sync.dma_start(out=table[t0 : t0 + msz, :], in_=ob[:msz, :])

    _finalize(nc)
    return nc


# ------------------------------------------------- edge schedule (host side)


def _edge_schedule(src, dst, ew, n_nodes, shard):
    """Partition edges by destination shard, sort by (dst tile, src block),
    build the SPMD-common gather/compute schedule (max counts over cores) and
    each core's index/weight streams laid into that skeleton.

    Returns (schedule, percore, dims).
    """
    blk = n_nodes // NBLK
    ntiles = (shard + 127) // 128
    core_of = dst // shard

    percore_edges = []
    cnt_all = np.zeros((NCORES, ntiles, NBLK), np.int64)
    for c in range(NCORES):
        m = core_of == c
        s = src[m]
        d = dst[m] - c * shard
        w = ew[m]
        tl = d // 128
        bl = s // blk
        order = np.lexsort((bl, tl))
        s, d, w, tl, bl = s[order], d[order], w[order], tl[order], bl[order]
        cnt = np.zeros((ntiles, NBLK), np.int64)
        np.add.at(cnt, (tl, bl), 1)
        cnt_all[c] = cnt
        run_off = np.zeros(ntiles * NBLK + 1, np.int64)
        np.cumsum(cnt.reshape(-1), out=run_off[1:])
        percore_edges.append((s, d, w, cnt, run_off))

    # SPMD skeleton: chunks per (tile, block) = ceil(max-over-cores / 128)
    nr_tb = (cnt_all.max(axis=0) + 127) // 128
    for t in range(ntiles):
        if nr_tb[t].sum() == 0:
            nr_tb[t, 0] = 1  # keep every tile non-empty

    nranges = (ntiles + R_TILES - 1) // R_TILES
    schedule = []
    icol_off = 0
    chunk_off = 0
    gmax = 1
    run_loc = {}  # (t, b) -> (icol, chunkcol, nchunks)
    for r in range(nranges):
        tlo, thi = r * R_TILES, min(ntiles, r * R_TILES + R_TILES)
        gathers = []
        for b in range(NBLK):
            nch_rb = int(nr_tb[tlo:thi, b].sum())
            if nch_rb == 0:
                continue
            ncalls = (nch_rb + GCAP - 1) // GCAP
            nch_pad = ncalls * GCAP
            off = 0
            for t in range(tlo, thi):
                if nr_tb[t, b]:
                    run_loc[(t, b)] = (
                        icol_off + off * 8,
                        chunk_off + off,
                        int(nr_tb[t, b]),
                    )
                    off += int(nr_tb[t, b])
            gathers.append(
                dict(
                    b=b,
                    icol=icol_off,
                    chunk0=chunk_off,
                    nchunks=nch_rb,
                    nch_pad=nch_pad,
                    ncalls=ncalls,
                )
            )
            gmax = max(gmax, nch_pad)
            icol_off += nch_pad * 8  # 128 idx per chunk = 8 cols of 16
            chunk_off += nch_rb
        tiles = []
        for t in range(tlo, thi):
            msz = min(128, shard - t * 128)
            runs = []
            for g in gathers:
                b = g["b"]
                if nr_tb[t, b]:
                    icol, chcol, nchk = run_loc[(t, b)]
                    runs.append((b, chcol - g["chunk0"], nchk, chcol))
            tiles.append(dict(t=t, msz=msz, runs=runs))
        schedule.append(dict(gathers=gathers, tiles=tiles))

    icols = max(icol_off, 16)
    tch = max(chunk_off, 1)

    percore = []
    for c in range(NCORES):
        s, d, w, cnt, run_off = percore_edges[c]
        idx_flat = np.zeros(tch * 128, np.int16)
        dst_flat = np.zeros(tch * 128, np.float32)
        w_flat = np.zeros(tch * 128, np.float32)
        for (t, b), (icol, chcol, nchk) in run_loc.items():
            n_real = int(cnt[t, b])
            if n_real == 0:
                continue
            i0 = int(run_off[t * NBLK + b])
            o0 = chcol * 128
            idx_flat[o0 : o0 + n_real] = (s[i0 : i0 + n_real] - b * blk).astype(
                np.int16
            )
            dst_flat[o0 : o0 + n_real] = (d[i0 : i0 + n_real] - t * 128).astype(
                np.float32
            )
            w_flat[o0 : o0 + n_real] = w[i0 : i0 + n_real]
        # lay real-chunk idx into the padded call skeleton; pad chunks gather
        # row 0 (finite, weight-0) so every call is a uniform full 512 idx
        idx_cols = np.zeros((icols // 8, 128), np.int16)
        for rng_ in schedule:
            for g in rng_["gathers"]:
                nch, c0 = g["nchunks"], g["chunk0"]
                block = idx_flat[c0 * 128 : (c0 + nch) * 128]
                base = g["icol"] // 8
                idx_cols[base : base + nch] = block.reshape(nch, 128)
        idx16 = np.tile(idx_cols.reshape(-1, 16).T, (8, 1))  # [128, icols]
        dstw = dst_flat.reshape(tch, 128).T.copy()
        wmat = w_flat.reshape(tch, 128).T.copy()
        percore.append(
            dict(
                idx=np.ascontiguousarray(idx16),
                dstw=np.ascontiguousarray(dstw),
                wmat=np.ascontiguousarray(wmat),
            )
        )

    fp = hash((nr_tb.tobytes(), shard, n_nodes))
    dims = dict(icols=icols, tch=tch, gmax=gmax, fingerprint=fp)
    return schedule, percore, dims


# ---------------------------------------------------------------- L2: edges


def _build_l2(n_nodes, ncls, shard, schedule, dims):
    blk = n_nodes // NBLK
    icols, tch, gmax = dims["icols"], dims["tch"], dims["gmax"]
    nc = bass.Bass(num_swdge_queues=NQUEUES, dynamic_dma_scratch_size=DMA_SCRATCH)
    table = nc.dram_tensor("table", [n_nodes, ROW], BF16, kind="ExternalInput")
    idxs = nc.dram_tensor("idxs", [128, icols], I16, kind="ExternalInput")
    dstw = nc.dram_tensor("dstw", [128, tch], BF16, kind="ExternalInput")
    wmat = nc.dram_tensor("wmat", [128, tch], BF16, kind="ExternalInput")
    b2t = nc.dram_tensor("b2t", [ncls, 1], F32, kind="ExternalInput")
    aggT = nc.dram_tensor("aggT", [ncls, shard], F32, kind="ExternalOutput")

    iota_np = np.tile(np.arange(128, dtype=np.float32), (128, 1))
    iota_t = nc.inline_tensor(iota_np, "iota")

    from contextlib import ExitStack

    with tile.TileContext(nc) as tc, ExitStack() as es:
        nidx_reg = es.enter_context(nc.gpsimd.register("nidx_reg"))
        with (
            tc.tile_pool(name="const", bufs=1) as constp,
            tc.tile_pool(name="idxp", bufs=2) as idxp,
            tc.tile_pool(name="gp", bufs=2) as gp,
            tc.tile_pool(name="ohp", bufs=4) as ohp,
            tc.tile_pool(name="evp", bufs=4) as evp,
            tc.tile_pool(name="psp", bufs=6, space="PSUM") as psp,
        ):
            nc.gpsimd.load_library(library_config.mlp)
            iota_f32 = constp.tile([128, 128], F32)
            nc.sync.dma_start(out=iota_f32[:], in_=iota_t[:])
            iota_s = constp.tile([128, 128], BF16)
            nc.vector.tensor_copy(iota_s[:], iota_f32[:])
            b2s = constp.tile([ncls, 1], F32)
            nc.sync.dma_start(out=b2s[:], in_=b2t[:])
            dstw_s = constp.tile([128, tch], BF16)
            nc.sync.dma_start(out=dstw_s[:], in_=dstw[:])
            wmat_s = constp.tile([128, tch], BF16)
            nc.sync.dma_start(out=wmat_s[:], in_=wmat[:])
            wmat_f = constp.tile([128, tch], F32)
            nc.vector.tensor_copy(wmat_f[:], wmat_s[:])

            nc.gpsimd.reg_mov(nidx_reg, GCAP * 128)
            qn = 0
            gcount = 0
            for rng in schedule:
                gathers = rng["gathers"]
                gbufs = {}
                if gathers:
                    icol0 = gathers[0]["icol"]
                    icoln = gathers[-1]["icol"] + gathers[-1]["nch_pad"] * 8
                    ib = idxp.tile([128, icoln - icol0], I16, tag="idx")
                    nc.sync.dma_start(out=ib[:], in_=idxs[:, icol0:icoln])
                for g in gathers:
                    b = g["b"]
                    gb = gp.tile([128, g["nch_pad"], ROW], BF16, tag=f"g{b}")
                    gbufs[b] = g
                    g["tile"] = gb
                    nch = g["nchunks"]
                    for k in range(g["ncalls"]):
                        c_lo = k * GCAP
                        ic = g["icol"] - icol0 + c_lo * 8
                        nc.gpsimd.dma_gather(
                            gb[:, c_lo : c_lo + GCAP, :],
                            table[b * blk : (b + 1) * blk, :],
                            ib[:, ic : ic + GCAP * 8],
                            GCAP * 128,
                            nidx_reg,
                            ROW,
                            single_packet=True,
                            queue_num=qn,
                        )
                        qn = (qn + 1) % NQUEUES
                    # scale messages by edge weight, split between the idle
                    # ACT engine (per-chunk Copy-with-scale) and DVE (batched)
                    c0g = g["chunk0"]
                    nact = int(round(nch * ACT_FRAC))
                    for j in range(nact):
                        nc.scalar.activation(
                            gb[:, j, :ncls],
                            gb[:, j, :ncls],
                            mybir.ActivationFunctionType.Copy,
                            bias=0.0,
                            scale=wmat_f[:, c0g + j : c0g + j + 1],
                        )
                    if nact < nch:
                        nc.vector.tensor_tensor(
                            gb[:, nact:nch, :ncls],
                            gb[:, nact:nch, :ncls],
                            wmat_s[:, c0g + nact : c0g + nch]
                            .unsqueeze(2)
                            .to_broadcast((128, nch - nact, ncls)),
                            mybir.AluOpType.mult,
                        )
                    gcount += 1
                    # one batched 0/1 one-hot build per gather group (its
                    # chunk columns are contiguous across the range's tiles)
                    oh = ohp.tile([128, nch, 128], BF16, tag="oh")
                    g["oh"] = oh
                    nc.vector.tensor_tensor(
                        oh[:],
                        dstw_s[:, c0g : c0g + nch]
                        .unsqueeze(2)
                        .to_broadcast((128, nch, 128)),
                        iota_s[:].unsqueeze(1).to_broadcast((128, nch, 128)),
                        mybir.AluOpType.is_equal,
                    )
                for tt in rng["tiles"]:
                    t, msz, runs = tt["t"], tt["msz"], tt["runs"]
                    ps = psp.tile([ncls, 128], F32, tag="ps")
                    nchunks_t = sum(nr for (_, _, nr, _) in runs)
                    ci = 0
                    for b, c0, nr, chcol in runs:
                        g = gbufs[b]
                        gb = g["tile"]
                        oh = g["oh"]
                        for j in range(nr):
                            nc.tensor.matmul(
                                ps[:, :msz],
                                gb[:, c0 + j, :ncls],
                                oh[:, c0 + j, :msz],
                                start=(ci == 0),
                                stop=(ci == nchunks_t - 1),
                            )
                            ci += 1
                    ev = evp.tile([ncls, 128], F32, tag="ev")
                    nc.vector.tensor_scalar_add(ev[:, :msz], ps[:, :msz], b2s[:])
                    nc.sync.dma_start(
                        out=aggT[:, t * 128 : t * 128 + msz], in_=ev[:, :msz]
                    )

    _finalize(nc)
    return nc


# ------------------------------------------------------------------- driver

_CACHE = {}
LAST_TIMES = {}


def _timed_run(name, nc, in_maps, core_ids):
    import time as _time

    t0 = _time.time()
    res = run_bass_kernel_spmd(nc, in_maps, core_ids)
    LAST_TIMES[name] = _time.time() - t0
    return res


def kernel(x, W1, b1, W2, b2, edge_index, edge_weight):
    x = np.asarray(x, np.float32)
    W1 = np.asarray(W1, np.float32)
    b1 = np.asarray(b1, np.float32)
    W2 = np.asarray(W2, np.float32)
    b2 = np.asarray(b2, np.float32)
    edge_index = np.asarray(edge_index)
    edge_weight = np.asarray(edge_weight, np.float32)

    n_nodes, nfeat = x.shape
    ncls = W2.shape[1]
    shard = n_nodes // NCORES
    core_ids = list(range(NCORES))

    # ---- L1: support table ----
    key1 = ("l1", n_nodes, nfeat, W1.shape[1], ncls)
    if key1 not in _CACHE:
        _CACHE[key1] = _build_l1(n_nodes, nfeat, W1.shape[1], ncls)
    nc1 = _CACHE[key1]

    xT = np.ascontiguousarray(x.T)
    in_maps1 = [
        {
            "xT": np.ascontiguousarray(xT[:, c * shard : (c + 1) * shard]),
            "W1": W1,
            "b1": np.ascontiguousarray(b1.reshape(-1, 1)),
            "W2": W2,
        }
        for c in core_ids
    ]
    res1 = _timed_run("l1", nc1, in_maps1, core_ids)
    table = np.ascontiguousarray(
        np.concatenate([res1.results[c]["table"] for c in core_ids], axis=0)
    )

    # ---- host edge preprocessing ----
    src = edge_index[0].astype(np.int64)
    dst = edge_index[1].astype(np.int64)
    ekey = ("sched", n_nodes, shard, edge_index.shape[1])
    if ekey in _CACHE and _CACHE[ekey][0] is not None:
        fph, schedule, percore, dims = _CACHE[ekey]
        if fph != hash(edge_index.tobytes()):
            schedule = None
    else:
        schedule = None
    if schedule is None:
        schedule, percore, dims = _edge_schedule(
            src, dst, edge_weight, n_nodes, shard
        )
        _CACHE[ekey] = (hash(edge_index.tobytes()), schedule, percore, dims)

    key2 = ("l2", n_nodes, ncls, shard, dims["fingerprint"])
    if key2 not in _CACHE:
        _CACHE[key2] = _build_l2(n_nodes, ncls, shard, schedule, dims)
    nc2 = _CACHE[key2]

    import ml_dtypes

    b2c = np.ascontiguousarray(b2.reshape(-1, 1))
    in_maps2 = [
        {
            "table": table,
            "idxs": percore[c]["idx"],
            "dstw": percore[c]["dstw"].astype(ml_dtypes.bfloat16),
            "wmat": percore[c]["wmat"].astype(ml_dtypes.bfloat16),
            "b2t": b2c,
        }
        for c in core_ids
    ]
    res2 = _timed_run("l2", nc2, in_maps2, core_ids)
    out = np.concatenate(
        [np.ascontiguousarray(res2.results[c]["aggT"].T.astype(np.float32)) for c in core_ids],
        axis=0,
    )
    return out


# revision 23
# speedup vs baseline: 43.2075x; 3.6168x over previous
"""GCN layer (x@W1 -> relu -> @W2 -> weighted scatter-add over edges) on 8 TRN2 cores.

Strategy (two launches, 8-way SPMD):
  L1: row-shard x across cores; each core computes its shard of
      support = relu(x@W1+b1)@W2 via TensorE (fp32 accumulate), writes its
      [N/8, 128] bf16 table shard (64 features + 64-byte pad -> 256B rows,
      the dma_gather minimum row size). Host concatenates shards.
  L2: edges partitioned by destination shard (core = dst // (N/8)).
      Per core: dma_gather (GPSIMD SWDGE, 4 queues, 512-idx calls, deep
      buffering) fetches 256B table rows per edge; DVE scales messages by
      edge weight (batched tensor_tensor) and builds 0/1 one-hot matrices
      (batched is_equal against an iota tile); TensorE accumulates
      agg.T[64, dst_tile] += msg.T @ onehot in PSUM; bias b2 added at PSUM
      evacuation. Host transposes/concats shards.

All floating-point math happens on device; the host only shards, sorts edge
indices, and concatenates outputs.
"""

import sys

if "/opt/trn_rl_repo" not in sys.path:
    sys.path.insert(0, "/opt/trn_rl_repo")

import numpy as np

import concourse.bass as bass
import concourse.tile as tile
from concourse import library_config, mybir
from concourse.bass_utils import run_bass_kernel_spmd
from concourse.library_overlay import lower_extended_insts

F32 = mybir.dt.float32
BF16 = mybir.dt.bfloat16
I16 = mybir.dt.int16

NCORES = 8
NBLK = 4  # int16 gather-index blocks (N/NBLK must be < 32768)
R_TILES = 4  # dst tiles per gather range
GCAP = 4  # chunks (x128 idx) per dma_gather call
NQUEUES = 4  # SWDGE queues
DMA_SCRATCH = 16384  # SWDGE descriptor-ring carveout
ACT_FRAC = 0.0  # fraction of each group's message-scales run on ACT (rest DVE)
ROW = 128  # bf16 elements per table row (256B: 64 features + 64 pad)

MAX_WAITS = 1  # this walrus build rejects >1 semaphore wait per instruction


def _split_excess_waits(nc, max_waits=MAX_WAITS):
    """Move excess sem-waits onto injected same-engine NOPs placed before the
    over-subscribed instruction (same-engine program order keeps semantics)."""
    uid = 0
    for f in nc.m.functions:
        for bb in f.blocks:
            il = bb.instructions
            new_il = []
            for inst in il:
                si = inst.sync_info
                waits = list(si.on_wait) if si and si.on_wait else []
                if len(waits) > max_waits:
                    excess, keep = waits[:-max_waits], waits[-max_waits:]
                    for j in range(0, len(excess), max_waits):
                        grp = excess[j : j + max_waits]
                        nop = mybir.InstNoOp(name=f"I-waitsplit-{uid}", ins=[], outs=[])
                        uid += 1
                        nop.engine = inst.engine
                        nop.sync_info = mybir.SyncInfo(on_wait=grp, on_update=[])
                        nc.register_instruction(nop, overwrite=True)
                        new_il.append(nop)
                    si.on_wait = keep
                new_il.append(inst)
            il[:] = new_il


def _finalize(nc):
    lower_extended_insts(nc)
    _split_excess_waits(nc)


# ---------------------------------------------------------------- L1: matmuls


def _build_l1(n_nodes, nfeat, nhid, ncls):
    shard = n_nodes // NCORES
    nc = bass.Bass()
    xT = nc.dram_tensor("xT", [nfeat, shard], BF16, kind="ExternalInput")
    W1 = nc.dram_tensor("W1", [nfeat, nhid], BF16, kind="ExternalInput")
    b1 = nc.dram_tensor("b1", [nhid, 1], F32, kind="ExternalInput")
    W2 = nc.dram_tensor("W2", [nhid, ncls], F32, kind="ExternalInput")
    table = nc.dram_tensor("table", [shard, ROW], BF16, kind="ExternalOutput")

    kchunks = nfeat // 128
    assert nfeat % 128 == 0 and nhid == 128
    NCHW = 512  # node columns per h chunk
    nch = (shard + NCHW - 1) // NCHW
    ntiles = (shard + 127) // 128

    with tile.TileContext(nc) as tc:
        with (
            tc.tile_pool(name="const", bufs=1) as constp,
            tc.tile_pool(name="xbuf", bufs=3) as xbuf,
            tc.tile_pool(name="hbuf", bufs=1) as hbuf,
            tc.tile_pool(name="obuf", bufs=4) as obuf,
            tc.tile_pool(name="psh", bufs=4, space="PSUM") as psh,
            tc.tile_pool(name="pss", bufs=4, space="PSUM") as pss,
        ):
            w1s = constp.tile([128, kchunks, nhid], BF16)
            nc.sync.dma_start(
                out=w1s[:], in_=W1[:].rearrange("(k p) h -> p k h", p=128)
            )
            w2s = constp.tile([128, ncls], F32)
            nc.sync.dma_start(out=w2s[:], in_=W2[:])
            b1s = constp.tile([128, 1], F32)
            nc.sync.dma_start(out=b1s[:], in_=b1[:])

            hT = hbuf.tile([128, shard], F32)  # resident h.T (fp32)
            for j in range(nch):
                j0 = j * NCHW
                nsz = min(NCHW, shard - j0)
                xt = xbuf.tile([128, kchunks, NCHW], BF16, tag="xt")
                nc.sync.dma_start(
                    out=xt[:, :, :nsz],
                    in_=xT[:, j0 : j0 + nsz].rearrange("(k p) n -> p k n", p=128),
                )
                ph = psh.tile([128, NCHW], F32)
                for k in range(kchunks):
                    nc.tensor.matmul(
                        ph[:, :nsz],
                        w1s[:, k, :],
                        xt[:, k, :nsz],
                        start=(k == 0),
                        stop=(k == kchunks - 1),
                    )
                nc.scalar.activation(
                    hT[:, j0 : j0 + nsz],
                    ph[:, :nsz],
                    mybir.ActivationFunctionType.Relu,
                    bias=b1s[:],
                    scale=1.0,
                )
            for t in range(ntiles):
                t0 = t * 128
                msz = min(128, shard - t0)
                ps = pss.tile([128, ncls], F32)
                nc.tensor.matmul(
                    ps[:msz, :], hT[:, t0 : t0 + msz], w2s[:], start=True, stop=True
                )
                ob = obuf.tile([128, ROW], BF16, tag="ob")
                nc.vector.memset(ob[:, ncls:], 0.0)
                nc.vector.tensor_copy(ob[:msz, :ncls], ps[:msz, :])
                nc.sync.dma_start(out=table[t0 : t0 + msz, :], in_=ob[:msz, :])

    _finalize(nc)
    return nc


# ------------------------------------------------- edge schedule (host side)


def _edge_schedule(src, dst, ew, n_nodes, shard):
    """Partition edges by destination shard, sort by (dst tile, src block),
    build the SPMD-common gather/compute schedule (max counts over cores) and
    each core's index/weight streams laid into that skeleton.

    Returns (schedule, percore, dims).
    """
    blk = n_nodes // NBLK
    ntiles = (shard + 127) // 128
    core_of = dst // shard

    percore_edges = []
    cnt_all = np.zeros((NCORES, ntiles, NBLK), np.int64)
    for c in range(NCORES):
        m = core_of == c
        s = src[m]
        d = dst[m] - c * shard
        w = ew[m]
        tl = d // 128
        bl = s // blk
        order = np.lexsort((bl, tl))
        s, d, w, tl, bl = s[order], d[order], w[order], tl[order], bl[order]
        cnt = np.zeros((ntiles, NBLK), np.int64)
        np.add.at(cnt, (tl, bl), 1)
        cnt_all[c] = cnt
        run_off = np.zeros(ntiles * NBLK + 1, np.int64)
        np.cumsum(cnt.reshape(-1), out=run_off[1:])
        percore_edges.append((s, d, w, cnt, run_off))

    # SPMD skeleton: chunks per (tile, block) = ceil(max-over-cores / 128)
    nr_tb = (cnt_all.max(axis=0) + 127) // 128
    for t in range(ntiles):
        if nr_tb[t].sum() == 0:
            nr_tb[t, 0] = 1  # keep every tile non-empty

    nranges = (ntiles + R_TILES - 1) // R_TILES
    schedule = []
    icol_off = 0
    chunk_off = 0
    gmax = 1
    run_loc = {}  # (t, b) -> (icol, chunkcol, nchunks)
    for r in range(nranges):
        tlo, thi = r * R_TILES, min(ntiles, r * R_TILES + R_TILES)
        gathers = []
        for b in range(NBLK):
            nch_rb = int(nr_tb[tlo:thi, b].sum())
            if nch_rb == 0:
                continue
            ncalls = (nch_rb + GCAP - 1) // GCAP
            nch_pad = ncalls * GCAP
            off = 0
            for t in range(tlo, thi):
                if nr_tb[t, b]:
                    run_loc[(t, b)] = (
                        icol_off + off * 8,
                        chunk_off + off,
                        int(nr_tb[t, b]),
                    )
                    off += int(nr_tb[t, b])
            gathers.append(
                dict(
                    b=b,
                    icol=icol_off,
                    chunk0=chunk_off,
                    nchunks=nch_rb,
                    nch_pad=nch_pad,
                    ncalls=ncalls,
                )
            )
            gmax = max(gmax, nch_pad)
            icol_off += nch_pad * 8  # 128 idx per chunk = 8 cols of 16
            chunk_off += nch_rb
        tiles = []
        for t in range(tlo, thi):
            msz = min(128, shard - t * 128)
            runs = []
            for g in gathers:
                b = g["b"]
                if nr_tb[t, b]:
                    icol, chcol, nchk = run_loc[(t, b)]
                    runs.append((b, chcol - g["chunk0"], nchk, chcol))
            tiles.append(dict(t=t, msz=msz, runs=runs))
        schedule.append(dict(gathers=gathers, tiles=tiles))

    icols = max(icol_off, 16)
    tch = max(chunk_off, 1)

    percore = []
    for c in range(NCORES):
        s, d, w, cnt, run_off = percore_edges[c]
        idx_flat = np.zeros(tch * 128, np.int16)
        dst_flat = np.zeros(tch * 128, np.float32)
        w_flat = np.zeros(tch * 128, np.float32)
        for (t, b), (icol, chcol, nchk) in run_loc.items():
            n_real = int(cnt[t, b])
            if n_real == 0:
                continue
            i0 = int(run_off[t * NBLK + b])
            o0 = chcol * 128
            idx_flat[o0 : o0 + n_real] = (s[i0 : i0 + n_real] - b * blk).astype(
                np.int16
            )
            dst_flat[o0 : o0 + n_real] = (d[i0 : i0 + n_real] - t * 128).astype(
                np.float32
            )
            w_flat[o0 : o0 + n_real] = w[i0 : i0 + n_real]
        # lay real-chunk idx into the padded call skeleton; pad chunks gather
        # row 0 (finite, weight-0) so every call is a uniform full 512 idx
        idx_cols = np.zeros((icols // 8, 128), np.int16)
        for rng_ in schedule:
            for g in rng_["gathers"]:
                nch, c0 = g["nchunks"], g["chunk0"]
                block = idx_flat[c0 * 128 : (c0 + nch) * 128]
                base = g["icol"] // 8
                idx_cols[base : base + nch] = block.reshape(nch, 128)
        idx16 = np.tile(idx_cols.reshape(-1, 16).T, (8, 1))  # [128, icols]
        dstw = dst_flat.reshape(tch, 128).T.copy()
        wmat = w_flat.reshape(tch, 128).T.copy()
        percore.append(
            dict(
                idx=np.ascontiguousarray(idx16),
                dstw=np.ascontiguousarray(dstw),
                wmat=np.ascontiguousarray(wmat),
            )
        )

    fp = hash((nr_tb.tobytes(), shard, n_nodes))
    dims = dict(icols=icols, tch=tch, gmax=gmax, fingerprint=fp)
    return schedule, percore, dims


# ---------------------------------------------------------------- L2: edges


def _build_l2(n_nodes, ncls, shard, schedule, dims):
    blk = n_nodes // NBLK
    icols, tch, gmax = dims["icols"], dims["tch"], dims["gmax"]
    nc = bass.Bass(num_swdge_queues=NQUEUES, dynamic_dma_scratch_size=DMA_SCRATCH)
    table = nc.dram_tensor("table", [n_nodes, ROW], BF16, kind="ExternalInput")
    idxs = nc.dram_tensor("idxs", [128, icols], I16, kind="ExternalInput")
    dstw = nc.dram_tensor("dstw", [128, tch], BF16, kind="ExternalInput")
    wmat = nc.dram_tensor("wmat", [128, tch], BF16, kind="ExternalInput")
    b2t = nc.dram_tensor("b2t", [ncls, 1], F32, kind="ExternalInput")
    aggT = nc.dram_tensor("aggT", [ncls, shard], F32, kind="ExternalOutput")

    iota_np = np.tile(np.arange(128, dtype=np.float32), (128, 1))
    iota_t = nc.inline_tensor(iota_np, "iota")

    from contextlib import ExitStack

    with tile.TileContext(nc) as tc, ExitStack() as es:
        nidx_reg = es.enter_context(nc.gpsimd.register("nidx_reg"))
        with (
            tc.tile_pool(name="const", bufs=1) as constp,
            tc.tile_pool(name="idxp", bufs=2) as idxp,
            tc.tile_pool(name="gp", bufs=2) as gp,
            tc.tile_pool(name="ohp", bufs=4) as ohp,
            tc.tile_pool(name="evp", bufs=4) as evp,
            tc.tile_pool(name="psp", bufs=6, space="PSUM") as psp,
        ):
            nc.gpsimd.load_library(library_config.mlp)
            iota_f32 = constp.tile([128, 128], F32)
            nc.sync.dma_start(out=iota_f32[:], in_=iota_t[:])
            iota_s = constp.tile([128, 128], BF16)
            nc.vector.tensor_copy(iota_s[:], iota_f32[:])
            b2s = constp.tile([ncls, 1], F32)
            nc.sync.dma_start(out=b2s[:], in_=b2t[:])
            dstw_s = constp.tile([128, tch], BF16)
            nc.sync.dma_start(out=dstw_s[:], in_=dstw[:])
            wmat_s = constp.tile([128, tch], BF16)
            nc.sync.dma_start(out=wmat_s[:], in_=wmat[:])
            wmat_f = constp.tile([128, tch], F32)
            nc.vector.tensor_copy(wmat_f[:], wmat_s[:])

            nc.gpsimd.reg_mov(nidx_reg, GCAP * 128)
            qn = 0
            gcount = 0
            for rng in schedule:
                gathers = rng["gathers"]
                gbufs = {}
                if gathers:
                    icol0 = gathers[0]["icol"]
                    icoln = gathers[-1]["icol"] + gathers[-1]["nch_pad"] * 8
                    ib = idxp.tile([128, icoln - icol0], I16, tag="idx")
                    nc.sync.dma_start(out=ib[:], in_=idxs[:, icol0:icoln])
                for g in gathers:
                    b = g["b"]
                    gb = gp.tile([128, g["nch_pad"], ROW], BF16, tag=f"g{b}")
                    gbufs[b] = g
                    g["tile"] = gb
                    nch = g["nchunks"]
                    for k in range(g["ncalls"]):
                        c_lo = k * GCAP
                        ic = g["icol"] - icol0 + c_lo * 8
                        nc.gpsimd.dma_gather(
                            gb[:, c_lo : c_lo + GCAP, :],
                            table[b * blk : (b + 1) * blk, :],
                            ib[:, ic : ic + GCAP * 8],
                            GCAP * 128,
                            nidx_reg,
                            ROW,
                            single_packet=True,
                            queue_num=qn,
                        )
                        qn = (qn + 1) % NQUEUES
                    # scale messages by edge weight, split between the idle
                    # ACT engine (per-chunk Copy-with-scale) and DVE (batched)
                    c0g = g["chunk0"]
                    nact = int(round(nch * ACT_FRAC))
                    for j in range(nact):
                        nc.scalar.activation(
                            gb[:, j, :ncls],
                            gb[:, j, :ncls],
                            mybir.ActivationFunctionType.Copy,
                            bias=0.0,
                            scale=wmat_f[:, c0g + j : c0g + j + 1],
                        )
                    if nact < nch:
                        nc.vector.tensor_tensor(
                            gb[:, nact:nch, :ncls],
                            gb[:, nact:nch, :ncls],
                            wmat_s[:, c0g + nact : c0g + nch]
                            .unsqueeze(2)
                            .to_broadcast((128, nch - nact, ncls)),
                            mybir.AluOpType.mult,
                        )
                    gcount += 1
                    # one batched 0/1 one-hot build per gather group (its
                    # chunk columns are contiguous across the range's tiles)
                    oh = ohp.tile([128, nch, 128], BF16, tag="oh")
                    g["oh"] = oh
                    nc.vector.tensor_tensor(
                        oh[:],
                        dstw_s[:, c0g : c0g + nch]
                        .unsqueeze(2)
                        .to_broadcast((128, nch, 128)),
                        iota_s[:].unsqueeze(1).to_broadcast((128, nch, 128)),
                        mybir.AluOpType.is_equal,
                    )
                for tt in rng["tiles"]:
                    t, msz, runs = tt["t"], tt["msz"], tt["runs"]
                    ps = psp.tile([ncls, 128], F32, tag="ps")
                    nchunks_t = sum(nr for (_, _, nr, _) in runs)
                    ci = 0
                    for b, c0, nr, chcol in runs:
                        g = gbufs[b]
                        gb = g["tile"]
                        oh = g["oh"]
                        for j in range(nr):
                            nc.tensor.matmul(
                                ps[:, :msz],
                                gb[:, c0 + j, :ncls],
                                oh[:, c0 + j, :msz],
                                start=(ci == 0),
                                stop=(ci == nchunks_t - 1),
                            )
                            ci += 1
                    ev = evp.tile([ncls, 128], F32, tag="ev")
                    nc.vector.tensor_scalar_add(ev[:, :msz], ps[:, :msz], b2s[:])
                    nc.sync.dma_start(
                        out=aggT[:, t * 128 : t * 128 + msz], in_=ev[:, :msz]
                    )

    _finalize(nc)
    return nc


# ------------------------------------------------------------------- driver

_CACHE = {}
LAST_TIMES = {}


def _timed_run(name, nc, in_maps, core_ids):
    import time as _time

    t0 = _time.time()
    res = run_bass_kernel_spmd(nc, in_maps, core_ids)
    LAST_TIMES[name] = _time.time() - t0
    return res


def kernel(x, W1, b1, W2, b2, edge_index, edge_weight):
    x = np.asarray(x, np.float32)
    W1 = np.asarray(W1, np.float32)
    b1 = np.asarray(b1, np.float32)
    W2 = np.asarray(W2, np.float32)
    b2 = np.asarray(b2, np.float32)
    edge_index = np.asarray(edge_index)
    edge_weight = np.asarray(edge_weight, np.float32)

    n_nodes, nfeat = x.shape
    ncls = W2.shape[1]
    shard = n_nodes // NCORES
    core_ids = list(range(NCORES))

    # ---- L1: support table ----
    key1 = ("l1", n_nodes, nfeat, W1.shape[1], ncls)
    if key1 not in _CACHE:
        _CACHE[key1] = _build_l1(n_nodes, nfeat, W1.shape[1], ncls)
    nc1 = _CACHE[key1]

    import ml_dtypes

    xT = np.ascontiguousarray(x.T).astype(ml_dtypes.bfloat16)
    W1b = W1.astype(ml_dtypes.bfloat16)
    in_maps1 = [
        {
            "xT": np.ascontiguousarray(xT[:, c * shard : (c + 1) * shard]),
            "W1": W1b,
            "b1": np.ascontiguousarray(b1.reshape(-1, 1)),
            "W2": W2,
        }
        for c in core_ids
    ]
    res1 = _timed_run("l1", nc1, in_maps1, core_ids)
    table = np.ascontiguousarray(
        np.concatenate([res1.results[c]["table"] for c in core_ids], axis=0)
    )

    # ---- host edge preprocessing ----
    src = edge_index[0].astype(np.int64)
    dst = edge_index[1].astype(np.int64)
    ekey = ("sched", n_nodes, shard, edge_index.shape[1])
    if ekey in _CACHE and _CACHE[ekey][0] is not None:
        fph, schedule, percore, dims = _CACHE[ekey]
        if fph != hash(edge_index.tobytes()):
            schedule = None
    else:
        schedule = None
    if schedule is None:
        schedule, percore, dims = _edge_schedule(
            src, dst, edge_weight, n_nodes, shard
        )
        _CACHE[ekey] = (hash(edge_index.tobytes()), schedule, percore, dims)

    key2 = ("l2", n_nodes, ncls, shard, dims["fingerprint"])
    if key2 not in _CACHE:
        _CACHE[key2] = _build_l2(n_nodes, ncls, shard, schedule, dims)
    nc2 = _CACHE[key2]

    import ml_dtypes

    b2c = np.ascontiguousarray(b2.reshape(-1, 1))
    in_maps2 = [
        {
            "table": table,
            "idxs": percore[c]["idx"],
            "dstw": percore[c]["dstw"].astype(ml_dtypes.bfloat16),
            "wmat": percore[c]["wmat"].astype(ml_dtypes.bfloat16),
            "b2t": b2c,
        }
        for c in core_ids
    ]
    res2 = _timed_run("l2", nc2, in_maps2, core_ids)
    out = np.concatenate(
        [np.ascontiguousarray(res2.results[c]["aggT"].T.astype(np.float32)) for c in core_ids],
        axis=0,
    )
    return out
